# revision 6
# baseline (speedup 1.0000x reference)
"""Multi-head attention (B=4, N=2048, DIM=768, H=8, DH=96) on 8 TRN2 NeuronCores.

Sharding: (batch, head)-parallel. Core c handles batch c//2 and the 4 heads
Hs = [0..3] (even c) or [4..7] (odd c) — selected host-side by weight slicing,
so the kernel is SPMD-identical. Each core computes K/V/Q for its 4 heads over
ALL 2048 tokens (no K/V duplication — the win vs data-parallel), runs
attention as 8 "virtual heads" vh = (head wh=vh%4, query-half: vh<4 = the
PARTNER's output half, vh>=4 = OWN half), then projects its OWN query half
using all 8 heads' normalized O: 4 local + 4 received from the pair core.

O-exchange: after each vh<4 completes, its normalized O^T [97,1024] bf16 is
ReduceScattered pairwise (replica_groups [[0,1],..]) with the payload
duplicated in both input shards; received = RS_out - own_payload (one DVE
subtract) recovers the partner's payload SPMD-symmetrically (~0.3% bf16
rounding noise). Collectives run on TOPSP/SDMA silicon and overlap freely
with compute. Staging/recv DMAs ride the otherwise-idle sync queue, and the
subtracts are deferred to vh7 slots 0-3 so no compute queue ever waits on
the collective round-trip (head-of-line blocking).

Per-core compute (all matmuls bf16, fp32 PSUM accumulation):
  - Dense Q^T/K^T projection per head (f-dim zero-padded 96->128), direct
    PSUM->SBUF eviction. K^T [128,2048] per head persists in SBUF (reused by
    both query halves).
  - V projection in natural space; slot 0 of each per-head V tile is a
    constant 1.0 column (1|V) so the attn@V matmul also produces the softmax
    row-sums in PSUM row 0.
  - dots P^T[nk,nq]: lhsT=K^T[96dh, 128nk], rhs=Q^T[96dh, 512nq]; softmax
    scale folded into w_q host-side. Two 512-query chunks land in one
    [128,1024] PSUM pair; ONE exp() on ScalarE PSUM->SBUF(bf16) covers both.
    No max subtraction (logits max ~9, fp32-safe).
  - O'^T[97, nq] accumulated over 16 key tiles; row 0 = row-sum s.
  - normalize: evacuate O' to SBUF, reciprocal of row 0, gpsimd
    partition_broadcast, single multiply (row 0 becomes 1.0 = the bias row).
  - proj y^T[c,nq] over 8 O-source slots (0-3 local own-half heads, 4-7
    received partner heads; host permutes wp to match): ct 0-2 accumulate
    slots {0,1,4,5,6,7} during vh6, ct 3-5 accumulate {0,1,2,4,5,6,7} during
    vh7 (bias folded into wp slot-0 row 0); missing slots added at the tail.

Input DMAs are split by first use across the sync and scalar HW DGE queues.
Output per core: y^T [768, 1024] bf16 (its own query half); host reassembles.
"""

import numpy as np
import ml_dtypes

B, N, DIM = 4, 2048, 768
H, DH = 8, 96
HL = 4             # local heads per core
NQ = N // 2        # query rows per core output
SCALE = DH ** -0.5
NCORES = 8
CT = DIM // 128    # 6 contraction chunks
NT = N // 128      # 16 key tiles
NQC = NQ // 512    # 2 query chunks of 512 per half
NVH = 8            # virtual heads = 4 local heads x 2 query halves

_CACHE = {}


def _build():
    import concourse.mybir as mybir
    import concourse.tile as tile
    from concourse import bacc

    f32 = mybir.dt.float32
    bf16 = mybir.dt.bfloat16
    Exp = mybir.ActivationFunctionType.Exp
    mult = mybir.AluOpType.mult
    sub = mybir.AluOpType.subtract

    nc = bacc.Bacc("TRN2", debug=False, num_devices=NCORES)

    xt_d = [
        nc.dram_tensor(f"xt{i}", [128, CT, 512], bf16, kind="ExternalInput")
        for i in range(4)
    ]
    wq0_d = nc.dram_tensor("wq0", [128, CT, 128], bf16, kind="ExternalInput")
    wk0_d = nc.dram_tensor("wk0", [128, CT, 128], bf16, kind="ExternalInput")
    wqr_d = nc.dram_tensor("wqr", [128, CT, (HL - 1) * 128], bf16, kind="ExternalInput")
    wkr_d = nc.dram_tensor("wkr", [128, CT, (HL - 1) * 128], bf16, kind="ExternalInput")
    wv_d = nc.dram_tensor("wv", [128, CT, HL * DH], bf16, kind="ExternalInput")
    wp_d = nc.dram_tensor("wp", [DH + 1, H, DIM], bf16, kind="ExternalInput")
    out_d = nc.dram_tensor("out", [DIM, NQ], bf16, kind="ExternalOutput")

    RG = [[0, 1], [2, 3], [4, 5], [6, 7]]

    with tile.TileContext(nc) as tc:
        with (
            tc.tile_pool(name="const", bufs=1) as cpool,
            tc.tile_pool(name="ptp", bufs=3) as pt_pool,
            tc.tile_pool(name="onp", bufs=14) as on_pool,
            tc.tile_pool(name="smallp", bufs=2) as small_pool,
            tc.tile_pool(name="stagep", bufs=3) as stage_pool,
            tc.tile_pool(name="ysb", bufs=4) as y_pool,
            tc.tile_pool(name="ccd", bufs=1, space="DRAM") as dram_pool,
            tc.tile_pool(name="ps_qkv", bufs=2, space="PSUM") as psum_qkv,
            tc.tile_pool(name="ps_d", bufs=2, space="PSUM") as psum_d,
            tc.tile_pool(name="ps_o", bufs=2, space="PSUM") as psum_o,
        ):
            # ---- persistent SBUF tensors, consolidated input DMAs ----
            xt_sb = [
                cpool.tile([128, CT, 512], bf16, name=f"xt_sb{i}") for i in range(4)
            ]
            wk0_sb = cpool.tile([128, CT, 128], bf16, name="wk0_sb")
            wq0_sb = cpool.tile([128, CT, 128], bf16, name="wq0_sb")
            wkr_sb = cpool.tile([128, CT, (HL - 1) * 128], bf16, name="wkr_sb")
            wqr_sb = cpool.tile([128, CT, (HL - 1) * 128], bf16, name="wqr_sb")
            wv_sb = cpool.tile([128, CT, HL * DH], bf16, name="wv_sb")
            wp_sb = cpool.tile([DH + 1, H, DIM], bf16, name="wp_sb")
            # K^T per local head, persistent (used by vh and vh+4)
            kt_sb = [cpool.tile([128, N], bf16, name=f"kt{w}") for w in range(HL)]
            qt_sb = {}   # vh -> [128, NQ] tile (rotating)
            v_sb = [cpool.tile([128, HL, DH + 1], bf16, name=f"v{t}") for t in range(NT)]
            y1_sb = [
                [cpool.tile([128, 512], bf16, name=f"y1_{ct}_{qc}") for qc in range(NQC)]
                for ct in range(CT)
            ]

            # inputs are pre-arranged partition-major on the host: every DMA
            # is a fully-contiguous per-partition transfer. Split across the
            # sync and scalar HW DGE queues, ordered by first use on each.
            nc.sync.dma_start(wk0_sb[:], wk0_d.ap())
            nc.scalar.dma_start(xt_sb[0][:], xt_d[0].ap())
            nc.sync.dma_start(wq0_sb[:], wq0_d.ap())
            nc.scalar.dma_start(xt_sb[1][:], xt_d[1].ap())
            nc.sync.dma_start(xt_sb[2][:], xt_d[2].ap())
            nc.scalar.dma_start(wv_sb[:], wv_d.ap())
            nc.sync.dma_start(xt_sb[3][:], xt_d[3].ap())
            nc.scalar.dma_start(wkr_sb[:], wkr_d.ap())
            nc.sync.dma_start(wqr_sb[:], wqr_d.ap())
            nc.scalar.dma_start(wp_sb[:], wp_d.ap())

            for t in range(NT):
                nc.vector.memset(v_sb[t][:, :, 0:1], 1.0)

            # identity for re-loading y1 into PSUM at the tail
            id_d = nc.inline_tensor(
                np.eye(128, dtype=ml_dtypes.bfloat16), name="id128"
            )
            id_sb = cpool.tile([128, 128], bf16, name="id_sb")
            nc.sync.dma_start(id_sb[:], id_d.ap())

            # PE warmup: keep the TensorEngine busy through the input-DMA
            # window so the HAM clock gate is at 8/8 when real work starts.
            warm_sb = cpool.tile([128, 128], bf16, name="warm_sb")
            nc.vector.memset(warm_sb[:], 0.0)
            ones_sb = cpool.tile([1, 128], f32, name="ones_sb")
            nc.vector.memset(ones_sb[:], 1.0)
            warm_ps = psum_qkv.tile([128, 128], f32, name="warmps", tag="qkvps")
            for _ in range(40):
                nc.tensor.matmul(
                    warm_ps, lhsT=warm_sb[:], rhs=warm_sb[:], start=True, stop=True
                )

            def xt_cols(lo):
                return xt_sb[lo // 512], lo % 512

            # ---- head-padded Q/K projection chunks (direct eviction) ----
            def k_chunk(wh, nc_):
                src, off = xt_cols(nc_ * 512)
                ps = psum_qkv.tile([128, 512], f32, name="kps", tag="qkvps")
                for ct in range(CT):
                    nc.tensor.matmul(
                        ps,
                        lhsT=(wk0_sb[:, ct, :] if wh == 0
                              else wkr_sb[:, ct, (wh - 1) * 128:wh * 128]),
                        rhs=src[:, ct, off:off + 512],
                        start=(ct == 0),
                        stop=(ct == CT - 1),
                    )
                nc.vector.tensor_copy(
                    out=kt_sb[wh][:, nc_ * 512:(nc_ + 1) * 512], in_=ps[:]
                )

            def q_chunk(vh, qc):
                if qc == 0:
                    qt_sb[vh] = stage_pool.tile(
                        [128, NQ], bf16, name="qt", tag="qt", bufs=3
                    )
                wh = vh % HL
                half = 0 if vh < HL else 1
                src, off = xt_cols(half * NQ + qc * 512)
                ps = psum_qkv.tile([128, 512], f32, name="qps", tag="qkvps")
                for ct in range(CT):
                    nc.tensor.matmul(
                        ps,
                        lhsT=(wq0_sb[:, ct, :] if wh == 0
                              else wqr_sb[:, ct, (wh - 1) * 128:wh * 128]),
                        rhs=src[:, ct, off:off + 512],
                        start=(ct == 0),
                        stop=(ct == CT - 1),
                    )
                nc.vector.tensor_copy(
                    out=qt_sb[vh][:, qc * 512:(qc + 1) * 512], in_=ps[:]
                )

            def v_chunk(t):
                src, off = xt_cols(t * 128)
                ps = psum_qkv.tile([128, 512], f32, name="vps", tag="qkvps")
                vps = ps[:, :HL * DH]
                for ct in range(CT):
                    nc.tensor.matmul(
                        vps,
                        lhsT=src[:, ct, off:off + 128],
                        rhs=wv_sb[:, ct, :],
                        start=(ct == 0),
                        stop=(ct == CT - 1),
                    )
                # single strided eviction into the 4 per-head [*,1:97] slots
                nc.vector.tensor_copy(
                    out=v_sb[t][:, :, 1:DH + 1],
                    in_=vps.rearrange("p (a b) -> p a b", a=HL),
                )

            # O-source slots for the projection: slot j<4 = local own-half
            # head j (vh 4+j), slot 4+w = received partner head (recv[w]).
            on_sb = {}
            rs_state = {}  # wh -> (rsum tile, pay tile)

            def proj_part(ct, qc):
                """Accumulate available slots; bias folded into slot 0."""
                yp = psum_qkv.tile([128, 512], f32, name="yps", tag="qkvps")
                srcs = [0, 1, 4, 5, 6, 7] if ct < 3 else [0, 1, 2, 4, 5, 6, 7]
                for i, j in enumerate(srcs):
                    nc.tensor.matmul(
                        yp,
                        lhsT=wp_sb[:, j, ct * 128:(ct + 1) * 128],
                        rhs=on_sb[(j, qc)][:],
                        start=(i == 0),
                        stop=(i == len(srcs) - 1),
                    )
                nc.vector.tensor_copy(out=y1_sb[ct][qc][:], in_=yp[:])

            def proj_tail(ct, use_scalar):
                # spread the six ct chunks over all three (now dead) PSUM
                # pools so the chains overlap instead of serializing on one
                # 2-buffer ring.
                tail_srcs = [2, 3] if ct < 3 else [3]
                y_sb = y_pool.tile([128, 1024], bf16, name="y", tag="y")
                if ct % 3 == 0:
                    yp = psum_d.tile([128, 1024], f32, name="yp7", tag="dps")
                    for qc in range(NQC):
                        yps = yp[:, qc * 512:(qc + 1) * 512]
                        nc.tensor.matmul(
                            yps, lhsT=id_sb[:], rhs=y1_sb[ct][qc][:],
                            start=True, stop=False,
                        )
                        for i, j in enumerate(tail_srcs):
                            nc.tensor.matmul(
                                yps,
                                lhsT=wp_sb[:, j, ct * 128:(ct + 1) * 128],
                                rhs=on_sb[(j, qc)][:],
                                start=False,
                                stop=(i == len(tail_srcs) - 1),
                            )
                    if use_scalar:
                        nc.scalar.copy(y_sb[:], yp[:])
                    else:
                        nc.vector.tensor_copy(out=y_sb[:], in_=yp[:])
                else:
                    pool, tag = (
                        (psum_qkv, "qkvps") if ct % 3 == 1 else (psum_o, "ops")
                    )
                    for qc in range(NQC):
                        yps = pool.tile([128, 512], f32, name="yp7n", tag=tag)
                        nc.tensor.matmul(
                            yps, lhsT=id_sb[:], rhs=y1_sb[ct][qc][:],
                            start=True, stop=False,
                        )
                        for i, j in enumerate(tail_srcs):
                            nc.tensor.matmul(
                                yps,
                                lhsT=wp_sb[:, j, ct * 128:(ct + 1) * 128],
                                rhs=on_sb[(j, qc)][:],
                                start=False,
                                stop=(i == len(tail_srcs) - 1),
                            )
                        half = y_sb[:, qc * 512:(qc + 1) * 512]
                        if qc == 0:
                            nc.scalar.copy(half, yps[:])
                        else:
                            nc.vector.tensor_copy(out=half, in_=yps[:])
                dma_eng = nc.sync if ct % 2 == 0 else nc.scalar
                dma_eng.dma_start(
                    out_d.ap()[ct * 128:(ct + 1) * 128, :], y_sb[:]
                )

            # ---- pairwise O-exchange: RS with duplicated payload ----
            # Staging + recv DMAs ride the sync queue (idle mid-kernel);
            # only the RS trigger itself sits on gpsimd. The subtract that
            # recovers the partner payload is deferred to vh7 (see below) so
            # the vector queue never waits on the collective round-trip.
            def exchange(wh, pay):
                in_b = dram_pool.tile(
                    [2 * (DH + 1), NQ], bf16, name=f"ccin{wh}", tag=f"ccin{wh}"
                )
                out_b = dram_pool.tile(
                    [DH + 1, NQ], bf16, name=f"ccout{wh}", tag=f"ccout{wh}"
                )
                nc.sync.dma_start(in_b[0:DH + 1, :], pay[:])
                nc.sync.dma_start(in_b[DH + 1:, :], pay[:])
                nc.gpsimd.collective_compute(
                    "ReduceScatter",
                    mybir.AluOpType.add,
                    replica_groups=RG,
                    ins=[in_b.opt()],
                    outs=[out_b.opt()],
                )
                rsum = on_pool.tile(
                    [DH + 1, NQ], bf16, name="rsum", tag=f"rsum{wh}", bufs=1
                )
                nc.sync.dma_start(rsum[:], out_b[:])
                rs_state[wh] = (rsum, pay)

            def finish_exchange(wh):
                rsum, pay = rs_state[wh]
                recv = on_pool.tile(
                    [DH + 1, NQ], bf16, name="recv", tag=f"recv{wh}", bufs=1
                )
                nc.vector.tensor_tensor(recv[:], rsum[:], pay[:], sub)
                for qc in range(NQC):
                    on_sb[(4 + wh, qc)] = recv[:, qc * 512:(qc + 1) * 512]

            # ---- attention for one virtual head, fillers interleaved ----
            def attn_head(vh, fillers):
                wh = vh % HL
                o_ps = [
                    psum_o.tile([DH + 1, 512], f32, name=f"ops{qc}", tag="ops")
                    for qc in range(NQC)
                ]
                for t in range(NT):
                    d_ps = psum_d.tile([128, 1024], f32, name="dps", tag="dps")
                    for qc in range(NQC):
                        nc.tensor.matmul(
                            d_ps[:, qc * 512:(qc + 1) * 512],
                            lhsT=kt_sb[wh][:, t * 128:(t + 1) * 128],
                            rhs=qt_sb[vh][:, qc * 512:(qc + 1) * 512],
                            start=True,
                            stop=True,
                        )
                    pt = pt_pool.tile([128, 1024], bf16, name="pt", tag="pt")
                    nc.scalar.activation(pt[:], d_ps[:], Exp)
                    for qc in range(NQC):
                        nc.tensor.matmul(
                            o_ps[qc],
                            lhsT=v_sb[t][:, wh, :],
                            rhs=pt[:, qc * 512:(qc + 1) * 512],
                            start=(t == 0),
                            stop=(t == NT - 1),
                        )
                    for fn in fillers.get(t, ()):
                        fn()
                # PSUM evacuation copies run immediately (frees the o_ps
                # banks); the rest of the normalize chain is returned as a
                # closure and emitted mid-way through the NEXT head, far from
                # the congested head boundary.
                o_sts = []
                for qc in range(NQC):
                    o_st = small_pool.tile(
                        [DH + 1, 512], f32, name="ostage", tag="ostage", bufs=7
                    )
                    nc.vector.tensor_copy(out=o_st[:], in_=o_ps[qc][:])
                    o_sts.append(o_st)

                def finish_normalize(vh=vh, wh=wh, o_sts=o_sts):
                    if vh == NVH - 1:
                        for qc in range(NQC):
                            # PE broadcast (fp32 matmul): sub-us vs 1us on
                            # gpsimd, and the PE is otherwise idle here
                            rsb = small_pool.tile(
                                [1, 512], f32, name="rsb", tag="rs", bufs=4
                            )
                            nc.vector.reciprocal_approx_fast(
                                out=rsb[:], in_=o_sts[qc][0:1, :]
                            )
                            bps = psum_o.tile(
                                [DH + 1, 512], f32, name="bps", tag="ops"
                            )
                            nc.tensor.matmul(
                                bps,
                                lhsT=ones_sb[0:1, 0:DH + 1],
                                rhs=rsb[:],
                                start=True,
                                stop=True,
                            )
                            on = on_pool.tile(
                                [DH + 1, 512], bf16, name="on", tag="on", bufs=2
                            )
                            on_sb[(3, qc)] = on
                            nc.vector.tensor_tensor(on[:], o_sts[qc][:], bps[:], mult)
                        return
                    # gpsimd partition_broadcast flavor; one [97,1024] tile
                    pay = on_pool.tile(
                        [DH + 1, NQ], bf16, name="onw", tag="onw", bufs=7
                    )
                    for qc in range(NQC):
                        rs = small_pool.tile([1, 512], f32, name="rs", tag="rs", bufs=4)
                        nc.vector.reciprocal_approx_fast(
                            out=rs[:], in_=o_sts[qc][0:1, :]
                        )
                        sb = small_pool.tile(
                            [DH + 1, 512], f32, name="sbc", tag="sbc", bufs=4
                        )
                        nc.gpsimd.partition_broadcast(sb[:], rs[:])
                        # row 0 becomes s*(1/s) = 1.0 -> the bias row
                        nc.vector.tensor_tensor(
                            pay[:, qc * 512:(qc + 1) * 512], o_sts[qc][:], sb[:], mult
                        )
                    if vh < HL:
                        # partner-destined: ship it
                        exchange(wh, pay)
                    else:
                        for qc in range(NQC):
                            on_sb[(vh - HL, qc)] = pay[:, qc * 512:(qc + 1) * 512]

                return finish_normalize

            # ---- software-pipelined emission ----
            k_chunk(0, 0)
            q_chunk(0, 0)
            q_chunk(0, 1)
            v_chunk(0)
            v_chunk(1)

            def mk_fillers(vh):
                f = {}

                def addf(slot, fn):
                    f.setdefault(slot, []).append(fn)

                if vh == 0:
                    for nc_ in (1, 2, 3):
                        addf(4 * nc_ - 3, (lambda n=nc_: k_chunk(0, n)))
                    for t in range(2, NT):
                        addf(t - 2, lambda tt=t: v_chunk(tt))
                # remaining heads' K during vh0-2 (kt persists; vh+4 reuses)
                if vh < HL - 1:
                    for i, nc_ in enumerate((0, 1, 2, 3)):
                        addf(2 * i + 2, (lambda hh=vh + 1, n=nc_: k_chunk(hh, n)))
                # next vh's Q
                if vh + 1 < NVH:
                    addf(11, lambda hh=vh + 1: q_chunk(hh, 0))
                    addf(13, lambda hh=vh + 1: q_chunk(hh, 1))
                # projection: ct 0-2 during vh6 (slots {0,1,4,5,6,7}),
                # ct 3-5 during vh7 (slots {0,1,2,4,5,6,7})
                if vh == NVH - 2:
                    # recover the received payloads first (all 4 RS results
                    # landed ~a window ago; zero wait on the vector queue)
                    for w in range(HL):
                        addf(w, lambda ww=w: finish_exchange(ww))
                    for i in range(6):
                        ct, qc = i // 2, i % 2
                        addf(2 * i + 5, lambda c=ct, q=qc: proj_part(c, q))
                if vh == NVH - 1:
                    for i in range(6):
                        ct, qc = 3 + i // 2, i % 2
                        addf(2 * i + 4, lambda c=ct, q=qc: proj_part(c, q))
                return f

            from collections import deque
            pending_norm = deque()
            for vh in range(NVH):
                f = mk_fillers(vh)
                if vh == NVH - 1:
                    # last head: all previous normalizes must land before
                    # the projection fillers (slots 4+)
                    while pending_norm:
                        f.setdefault(2, []).append(pending_norm.popleft())
                elif pending_norm:
                    f.setdefault(3, []).insert(0, pending_norm.popleft())
                pending_norm.append(attn_head(vh, f))

            # ---- tail: vh7 normalize + remaining-slot projection + out ----
            # keep the PE warm through the normalize window so the proj_tail
            # matmuls run at full clock: dummy matmuls gated on vh7's
            # evacuated O' (so the scheduler can't hoist them earlier).
            h7_norm = pending_norm.popleft()
            for _ in range(10):
                wps = psum_o.tile([DH + 1, 512], f32, name="wps", tag="ops")
                nc.tensor.matmul(
                    wps,
                    lhsT=warm_sb[0:DH + 1, 0:DH + 1],
                    rhs=qt_sb[NVH - 1][0:DH + 1, 0:512],
                    start=True,
                    stop=True,
                )
            h7_norm()
            for ct in range(CT):
                proj_tail(ct, use_scalar=(ct % 2 == 1))

    nc.compile()
    return nc


def _get_nc():
    if "nc" not in _CACHE:
        _CACHE["nc"] = _build()
    return _CACHE["nc"]


def _prep_shards(x, w_qkv, w_proj, b_proj):
    bf16 = ml_dtypes.bfloat16
    x = np.asarray(x, dtype=np.float32)
    w_qkv = np.asarray(w_qkv, dtype=np.float32)
    w_proj = np.asarray(w_proj, dtype=np.float32)
    b_proj = np.asarray(b_proj, dtype=np.float32)

    def pmajor(w):  # [768(c), F] -> [128, CT, F] partition-major contiguous
        return np.ascontiguousarray(
            w.reshape(CT, 128, w.shape[1]).transpose(1, 0, 2)
        ).astype(bf16)

    def pad_heads4(w, heads):  # [768(c), 768(f)] -> [768, 4*128] zero-padded
        wp_ = np.zeros((DIM, HL, 128), np.float32)
        wp_[:, :, :DH] = w.reshape(DIM, H, DH)[:, heads, :]
        return wp_.reshape(DIM, HL * 128)

    wq_t = w_qkv[0:DIM].T * SCALE           # [768(c), 768(f)]
    wk_t = w_qkv[DIM:2 * DIM].T
    wv_t = w_qkv[2 * DIM:3 * DIM].T
    wp_heads = w_proj.T.reshape(H, DH, DIM)  # [H, DH, DIM]

    in_maps = []
    for c in range(NCORES):
        b, parity = divmod(c, 2)
        Hs = list(range(0, HL)) if parity == 0 else list(range(HL, H))
        Ho = list(range(HL, H)) if parity == 0 else list(range(0, HL))

        xt = x[b].T  # [768, 2048]
        # arrange columns: [0:1024] = PARTNER's output half, [1024:2048] = OWN
        if parity == 0:
            xt = np.concatenate([xt[:, NQ:], xt[:, :NQ]], axis=1)
        wq_b = pmajor(pad_heads4(wq_t, Hs))
        wk_b = pmajor(pad_heads4(wk_t, Hs))
        wv_b = pmajor(
            np.ascontiguousarray(
                wv_t.reshape(DIM, H, DH)[:, Hs, :]
            ).reshape(DIM, HL * DH)
        )
        # wp slots: j<4 = own head Hs[j], j>=4 = partner head Ho[j-4];
        # row 0 = bias (slot 0 only), rows 1..DH = weights
        wp_arr = np.zeros((DH + 1, H, DIM), np.float32)
        for j, hh in enumerate(Hs + Ho):
            wp_arr[1:DH + 1, j, :] = wp_heads[hh]
        wp_arr[0, 0, :] = b_proj
        m = {
            f"xt{i}": pmajor(xt[:, i * 512:(i + 1) * 512]) for i in range(4)
        }
        m.update({
            "wq0": np.ascontiguousarray(wq_b[:, :, 0:128]),
            "wqr": np.ascontiguousarray(wq_b[:, :, 128:]),
            "wk0": np.ascontiguousarray(wk_b[:, :, 0:128]),
            "wkr": np.ascontiguousarray(wk_b[:, :, 128:]),
            "wv": wv_b,
            "wp": np.ascontiguousarray(wp_arr).astype(bf16),
        })
        in_maps.append(m)
    return in_maps


def kernel(x, w_qkv, w_proj, b_proj):
    from concourse.bass_utils import run_bass_kernel_spmd

    nc = _get_nc()
    in_maps = _prep_shards(x, w_qkv, w_proj, b_proj)
    res = run_bass_kernel_spmd(nc, in_maps, core_ids=list(range(NCORES)))
    out = np.empty((B, N, DIM), np.float32)
    for c in range(NCORES):
        b, half = divmod(c, 2)
        yT = np.asarray(res.results[c]["out"], dtype=np.float32)  # [768, 1024]
        out[b, half * NQ:(half + 1) * NQ, :] = yT.T
    return out


# revision 8
# speedup vs baseline: 1.0142x; 1.0142x over previous
"""Multi-head attention (B=4, N=2048, DIM=768, H=8, DH=96) on 8 TRN2 NeuronCores.

Sharding: (batch, head)-parallel. Core c handles batch c//2 and the 4 heads
Hs = [0..3] (even c) or [4..7] (odd c) — selected host-side by weight slicing,
so the kernel is SPMD-identical. Each core computes K/V/Q for its 4 heads over
ALL 2048 tokens (no K/V duplication — the win vs data-parallel), runs
attention as 8 "virtual heads" vh = (head wh=vh%4, query-half: vh<4 = the
PARTNER's output half, vh>=4 = OWN half), then projects its OWN query half
using all 8 heads' normalized O: 4 local + 4 received from the pair core.

O-exchange: ONE pairwise ReduceScatter (replica_groups [[0,1],..]) of a
[2x388, 1024] bf16 buffer. The SPMD read/write asymmetry is resolved by
host-provided 0/1 masks folded into the softmax normalization: each payload
head emits payA = O*(1/s * mA) and payB = O*(1/s * mB) with (mA,mB) = (0,1)
on even cores and (1,0) on odd cores. Shard A rows carry the odd core's
payload, shard B the even core's, so the RS sum is EXACTLY the partner's
payload on both sides (own side contributes zeros) — no subtract, no
rounding noise, and no compute queue ever waits on the collective (the
recv DMAs ride the otherwise-idle sync queue; only the vh6/vh7 projection
reads the result, ~2 windows after the RS completes).

Per-core compute (all matmuls bf16, fp32 PSUM accumulation):
  - Dense Q^T/K^T projection per head (f-dim zero-padded 96->128), direct
    PSUM->SBUF eviction. K^T [128,2048] per head persists in SBUF (reused by
    both query halves).
  - V projection in natural space; slot 0 of each per-head V tile is a
    constant 1.0 column (1|V) so the attn@V matmul also produces the softmax
    row-sums in PSUM row 0.
  - dots P^T[nk,nq]: lhsT=K^T[96dh, 128nk], rhs=Q^T[96dh, 512nq]; softmax
    scale folded into w_q host-side. Two 512-query chunks land in one
    [128,1024] PSUM pair; ONE exp() on ScalarE PSUM->SBUF(bf16) covers both.
    No max subtraction (logits max ~9, fp32-safe).
  - O'^T[97, nq] accumulated over 16 key tiles; row 0 = row-sum s.
  - normalize: evacuate O' to SBUF, reciprocal of row 0 (DVE), PE-broadcast
    (ones[1,97] matmul — keeps gpsimd's queue empty so the RS trigger can
    block it harmlessly), multiply (row 0 becomes mA/mB/1.0 = the bias row).
  - proj y^T[c,nq] over 8 O-source slots (0-3 local own-half heads, 4-7
    received partner heads; host permutes wp to match): ct 0-2 accumulate
    slots {0,1,4,5,6,7} during vh6, ct 3-5 accumulate {0,1,2,4,5,6,7} during
    vh7 (bias folded into wp slot-0 row 0); missing slots added at the tail.

Input DMAs are split by first use across the sync and scalar HW DGE queues.
Output per core: y^T [768, 1024] bf16 (its own query half); host reassembles.
"""

import numpy as np
import ml_dtypes

B, N, DIM = 4, 2048, 768
H, DH = 8, 96
HL = 4             # local heads per core
NQ = N // 2        # query rows per core output
SCALE = DH ** -0.5
NCORES = 8
CT = DIM // 128    # 6 contraction chunks
NT = N // 128      # 16 key tiles
NQC = NQ // 512    # 2 query chunks of 512 per half
NVH = 8            # virtual heads = 4 local heads x 2 query halves

_CACHE = {}


def _build():
    import concourse.mybir as mybir
    import concourse.tile as tile
    from concourse import bacc

    f32 = mybir.dt.float32
    bf16 = mybir.dt.bfloat16
    Exp = mybir.ActivationFunctionType.Exp
    mult = mybir.AluOpType.mult

    nc = bacc.Bacc("TRN2", debug=False, num_devices=NCORES)

    xt_d = [
        nc.dram_tensor(f"xt{i}", [128, CT, 512], bf16, kind="ExternalInput")
        for i in range(4)
    ]
    wq0_d = nc.dram_tensor("wq0", [128, CT, 128], bf16, kind="ExternalInput")
    wk0_d = nc.dram_tensor("wk0", [128, CT, 128], bf16, kind="ExternalInput")
    wqr_d = nc.dram_tensor("wqr", [128, CT, (HL - 1) * 128], bf16, kind="ExternalInput")
    wkr_d = nc.dram_tensor("wkr", [128, CT, (HL - 1) * 128], bf16, kind="ExternalInput")
    wv_d = nc.dram_tensor("wv", [128, CT, HL * DH], bf16, kind="ExternalInput")
    wp_d = nc.dram_tensor("wp", [DH + 1, H, DIM], bf16, kind="ExternalInput")
    mka_d = nc.dram_tensor("mka", [1, 512], f32, kind="ExternalInput")
    mkb_d = nc.dram_tensor("mkb", [1, 512], f32, kind="ExternalInput")
    out_d = nc.dram_tensor("out", [DIM, NQ], bf16, kind="ExternalOutput")

    RG = [[0, 1], [2, 3], [4, 5], [6, 7]]

    with tile.TileContext(nc) as tc:
        with (
            tc.tile_pool(name="const", bufs=1) as cpool,
            tc.tile_pool(name="ptp", bufs=3) as pt_pool,
            tc.tile_pool(name="onp", bufs=14) as on_pool,
            tc.tile_pool(name="smallp", bufs=2) as small_pool,
            tc.tile_pool(name="stagep", bufs=3) as stage_pool,
            tc.tile_pool(name="ysb", bufs=4) as y_pool,
            tc.tile_pool(name="ccd", bufs=1, space="DRAM") as dram_pool,
            tc.tile_pool(name="ps_qkv", bufs=2, space="PSUM") as psum_qkv,
            tc.tile_pool(name="ps_d", bufs=2, space="PSUM") as psum_d,
            tc.tile_pool(name="ps_o", bufs=2, space="PSUM") as psum_o,
        ):
            # ---- persistent SBUF tensors, consolidated input DMAs ----
            xt_sb = [
                cpool.tile([128, CT, 512], bf16, name=f"xt_sb{i}") for i in range(4)
            ]
            wk0_sb = cpool.tile([128, CT, 128], bf16, name="wk0_sb")
            wq0_sb = cpool.tile([128, CT, 128], bf16, name="wq0_sb")
            wkr_sb = cpool.tile([128, CT, (HL - 1) * 128], bf16, name="wkr_sb")
            wqr_sb = cpool.tile([128, CT, (HL - 1) * 128], bf16, name="wqr_sb")
            wv_sb = cpool.tile([128, CT, HL * DH], bf16, name="wv_sb")
            wp_sb = cpool.tile([DH + 1, H, DIM], bf16, name="wp_sb")
            mka_sb = cpool.tile([1, 512], f32, name="mka_sb")
            mkb_sb = cpool.tile([1, 512], f32, name="mkb_sb")
            # K^T per local head, persistent (used by vh and vh+4)
            kt_sb = [cpool.tile([128, N], bf16, name=f"kt{w}") for w in range(HL)]
            qt_sb = {}   # vh -> [128, NQ] tile (rotating)
            v_sb = [cpool.tile([128, HL, DH + 1], bf16, name=f"v{t}") for t in range(NT)]
            y1_sb = [
                [cpool.tile([128, 512], bf16, name=f"y1_{ct}_{qc}") for qc in range(NQC)]
                for ct in range(CT)
            ]

            # collective bounce buffers: shard A rows = payA (odd core's
            # payload), shard B rows = payB (even core's); RS(add) -> the
            # partner's payload, exactly.
            in_b = dram_pool.tile([2 * HL * (DH + 1), NQ], bf16, name="ccin", tag="ccin")
            out_b = dram_pool.tile(
                [HL * (DH + 1), NQ], bf16, name="ccout", tag="ccout"
            )

            # inputs are pre-arranged partition-major on the host: every DMA
            # is a fully-contiguous per-partition transfer. Split across the
            # sync and scalar HW DGE queues, ordered by first use on each.
            nc.sync.dma_start(wk0_sb[:], wk0_d.ap())
            nc.scalar.dma_start(xt_sb[0][:], xt_d[0].ap())
            nc.sync.dma_start(wq0_sb[:], wq0_d.ap())
            nc.scalar.dma_start(xt_sb[1][:], xt_d[1].ap())
            nc.sync.dma_start(xt_sb[2][:], xt_d[2].ap())
            nc.scalar.dma_start(wv_sb[:], wv_d.ap())
            nc.sync.dma_start(xt_sb[3][:], xt_d[3].ap())
            nc.scalar.dma_start(wkr_sb[:], wkr_d.ap())
            nc.sync.dma_start(wqr_sb[:], wqr_d.ap())
            nc.scalar.dma_start(wp_sb[:], wp_d.ap())
            nc.sync.dma_start(mka_sb[:], mka_d.ap())
            nc.sync.dma_start(mkb_sb[:], mkb_d.ap())

            for t in range(NT):
                nc.vector.memset(v_sb[t][:, :, 0:1], 1.0)

            # identity for re-loading y1 into PSUM at the tail
            id_d = nc.inline_tensor(
                np.eye(128, dtype=ml_dtypes.bfloat16), name="id128"
            )
            id_sb = cpool.tile([128, 128], bf16, name="id_sb")
            nc.sync.dma_start(id_sb[:], id_d.ap())

            # PE warmup: keep the TensorEngine busy through the input-DMA
            # window so the HAM clock gate is at 8/8 when real work starts.
            warm_sb = cpool.tile([128, 128], bf16, name="warm_sb")
            nc.vector.memset(warm_sb[:], 0.0)
            ones_sb = cpool.tile([1, 128], f32, name="ones_sb")
            nc.vector.memset(ones_sb[:], 1.0)
            warm_ps = psum_qkv.tile([128, 128], f32, name="warmps", tag="qkvps")
            for _ in range(40):
                nc.tensor.matmul(
                    warm_ps, lhsT=warm_sb[:], rhs=warm_sb[:], start=True, stop=True
                )

            def xt_cols(lo):
                return xt_sb[lo // 512], lo % 512

            # ---- head-padded Q/K projection chunks (direct eviction) ----
            def k_chunk(wh, nc_):
                src, off = xt_cols(nc_ * 512)
                ps = psum_qkv.tile([128, 512], f32, name="kps", tag="qkvps")
                for ct in range(CT):
                    nc.tensor.matmul(
                        ps,
                        lhsT=(wk0_sb[:, ct, :] if wh == 0
                              else wkr_sb[:, ct, (wh - 1) * 128:wh * 128]),
                        rhs=src[:, ct, off:off + 512],
                        start=(ct == 0),
                        stop=(ct == CT - 1),
                    )
                nc.vector.tensor_copy(
                    out=kt_sb[wh][:, nc_ * 512:(nc_ + 1) * 512], in_=ps[:]
                )

            def q_chunk(vh, qc):
                if qc == 0:
                    qt_sb[vh] = stage_pool.tile(
                        [128, NQ], bf16, name="qt", tag="qt", bufs=3
                    )
                wh = vh % HL
                half = 0 if vh < HL else 1
                src, off = xt_cols(half * NQ + qc * 512)
                ps = psum_qkv.tile([128, 512], f32, name="qps", tag="qkvps")
                for ct in range(CT):
                    nc.tensor.matmul(
                        ps,
                        lhsT=(wq0_sb[:, ct, :] if wh == 0
                              else wqr_sb[:, ct, (wh - 1) * 128:wh * 128]),
                        rhs=src[:, ct, off:off + 512],
                        start=(ct == 0),
                        stop=(ct == CT - 1),
                    )
                nc.vector.tensor_copy(
                    out=qt_sb[vh][:, qc * 512:(qc + 1) * 512], in_=ps[:]
                )

            def v_chunk(t):
                src, off = xt_cols(t * 128)
                ps = psum_qkv.tile([128, 512], f32, name="vps", tag="qkvps")
                vps = ps[:, :HL * DH]
                for ct in range(CT):
                    nc.tensor.matmul(
                        vps,
                        lhsT=src[:, ct, off:off + 128],
                        rhs=wv_sb[:, ct, :],
                        start=(ct == 0),
                        stop=(ct == CT - 1),
                    )
                # single strided eviction into the 4 per-head [*,1:97] slots
                nc.vector.tensor_copy(
                    out=v_sb[t][:, :, 1:DH + 1],
                    in_=vps.rearrange("p (a b) -> p a b", a=HL),
                )

            # O-source slots for the projection: slot j<4 = local own-half
            # head j (vh 4+j), slot 4+w = received partner head (recv[w]).
            on_sb = {}

            def pe_broadcast(rs):
                """[1,512] -> [97,512] PSUM via ones-matmul on the PE."""
                bps = psum_qkv.tile([DH + 1, 512], f32, name="bps", tag="qkvps")
                nc.tensor.matmul(
                    bps, lhsT=ones_sb[0:1, 0:DH + 1], rhs=rs[:],
                    start=True, stop=True,
                )
                return bps

            def proj_part(ct, qc):
                """Accumulate available slots; bias folded into slot 0."""
                yp = psum_qkv.tile([128, 512], f32, name="yps", tag="qkvps")
                srcs = [0, 1, 4, 5, 6, 7] if ct < 3 else [0, 1, 2, 4, 5, 6, 7]
                for i, j in enumerate(srcs):
                    nc.tensor.matmul(
                        yp,
                        lhsT=wp_sb[:, j, ct * 128:(ct + 1) * 128],
                        rhs=on_sb[(j, qc)][:],
                        start=(i == 0),
                        stop=(i == len(srcs) - 1),
                    )
                nc.vector.tensor_copy(out=y1_sb[ct][qc][:], in_=yp[:])

            def proj_tail(ct, use_scalar):
                # spread the six ct chunks over all three (now dead) PSUM
                # pools so the chains overlap instead of serializing on one
                # 2-buffer ring.
                tail_srcs = [2, 3] if ct < 3 else [3]
                y_sb = y_pool.tile([128, 1024], bf16, name="y", tag="y")
                if ct % 3 == 0:
                    yp = psum_d.tile([128, 1024], f32, name="yp7", tag="dps")
                    for qc in range(NQC):
                        yps = yp[:, qc * 512:(qc + 1) * 512]
                        nc.tensor.matmul(
                            yps, lhsT=id_sb[:], rhs=y1_sb[ct][qc][:],
                            start=True, stop=False,
                        )
                        for i, j in enumerate(tail_srcs):
                            nc.tensor.matmul(
                                yps,
                                lhsT=wp_sb[:, j, ct * 128:(ct + 1) * 128],
                                rhs=on_sb[(j, qc)][:],
                                start=False,
                                stop=(i == len(tail_srcs) - 1),
                            )
                    if use_scalar:
                        nc.scalar.copy(y_sb[:], yp[:])
                    else:
                        nc.vector.tensor_copy(out=y_sb[:], in_=yp[:])
                else:
                    pool, tag = (
                        (psum_qkv, "qkvps") if ct % 3 == 1 else (psum_o, "ops")
                    )
                    for qc in range(NQC):
                        yps = pool.tile([128, 512], f32, name="yp7n", tag=tag)
                        nc.tensor.matmul(
                            yps, lhsT=id_sb[:], rhs=y1_sb[ct][qc][:],
                            start=True, stop=False,
                        )
                        for i, j in enumerate(tail_srcs):
                            nc.tensor.matmul(
                                yps,
                                lhsT=wp_sb[:, j, ct * 128:(ct + 1) * 128],
                                rhs=on_sb[(j, qc)][:],
                                start=False,
                                stop=(i == len(tail_srcs) - 1),
                            )
                        half = y_sb[:, qc * 512:(qc + 1) * 512]
                        if qc == 0:
                            nc.scalar.copy(half, yps[:])
                        else:
                            nc.vector.tensor_copy(out=half, in_=yps[:])
                dma_eng = nc.sync if ct % 2 == 0 else nc.scalar
                dma_eng.dma_start(
                    out_d.ap()[ct * 128:(ct + 1) * 128, :], y_sb[:]
                )

            # ---- attention for one virtual head, fillers interleaved ----
            def attn_head(vh, fillers):
                wh = vh % HL
                o_ps = [
                    psum_o.tile([DH + 1, 512], f32, name=f"ops{qc}", tag="ops")
                    for qc in range(NQC)
                ]
                for t in range(NT):
                    d_ps = psum_d.tile([128, 1024], f32, name="dps", tag="dps")
                    for qc in range(NQC):
                        nc.tensor.matmul(
                            d_ps[:, qc * 512:(qc + 1) * 512],
                            lhsT=kt_sb[wh][:, t * 128:(t + 1) * 128],
                            rhs=qt_sb[vh][:, qc * 512:(qc + 1) * 512],
                            start=True,
                            stop=True,
                        )
                    pt = pt_pool.tile([128, 1024], bf16, name="pt", tag="pt")
                    nc.scalar.activation(pt[:], d_ps[:], Exp)
                    for qc in range(NQC):
                        nc.tensor.matmul(
                            o_ps[qc],
                            lhsT=v_sb[t][:, wh, :],
                            rhs=pt[:, qc * 512:(qc + 1) * 512],
                            start=(t == 0),
                            stop=(t == NT - 1),
                        )
                    for fn in fillers.get(t, ()):
                        fn()
                # PSUM evacuation copies run immediately (frees the o_ps
                # banks); the rest of the normalize chain is returned as a
                # closure and emitted mid-way through the NEXT head, far from
                # the congested head boundary.
                o_sts = []
                for qc in range(NQC):
                    o_st = small_pool.tile(
                        [DH + 1, 512], f32, name="ostage", tag="ostage", bufs=7
                    )
                    nc.vector.tensor_copy(out=o_st[:], in_=o_ps[qc][:])
                    o_sts.append(o_st)

                def finish_normalize(vh=vh, wh=wh, o_sts=o_sts):
                    if vh < HL:
                        # partner-destined: masked dual payload, then stage
                        payA = on_pool.tile(
                            [DH + 1, NQ], bf16, name="payA", tag="payA", bufs=4
                        )
                        payB = on_pool.tile(
                            [DH + 1, NQ], bf16, name="payB", tag="payB", bufs=4
                        )
                        for qc in range(NQC):
                            rs = small_pool.tile(
                                [1, 512], f32, name="rs", tag="rs", bufs=4
                            )
                            nc.vector.reciprocal_approx_fast(
                                out=rs[:], in_=o_sts[qc][0:1, :]
                            )
                            rsA = small_pool.tile(
                                [1, 512], f32, name="rsA", tag="rsA", bufs=4
                            )
                            rsB = small_pool.tile(
                                [1, 512], f32, name="rsB", tag="rsB", bufs=4
                            )
                            nc.vector.tensor_tensor(rsA[:], rs[:], mka_sb[:], mult)
                            nc.vector.tensor_tensor(rsB[:], rs[:], mkb_sb[:], mult)
                            cs = qc * 512
                            nc.vector.tensor_tensor(
                                payA[:, cs:cs + 512], o_sts[qc][:],
                                pe_broadcast(rsA)[:], mult,
                            )
                            nc.vector.tensor_tensor(
                                payB[:, cs:cs + 512], o_sts[qc][:],
                                pe_broadcast(rsB)[:], mult,
                            )
                        r0 = wh * (DH + 1)
                        nc.sync.dma_start(in_b[r0:r0 + DH + 1, :], payA[:])
                        r1 = HL * (DH + 1) + r0
                        nc.sync.dma_start(in_b[r1:r1 + DH + 1, :], payB[:])
                        if wh == HL - 1:
                            # all 8 payload blocks staged -> ONE collective
                            nc.gpsimd.collective_compute(
                                "ReduceScatter",
                                mybir.AluOpType.add,
                                replica_groups=RG,
                                ins=[in_b.opt()],
                                outs=[out_b.opt()],
                            )
                            for w in range(HL):
                                recv = on_pool.tile(
                                    [DH + 1, NQ], bf16,
                                    name="recv", tag=f"recv{w}", bufs=1,
                                )
                                rw = w * (DH + 1)
                                nc.sync.dma_start(recv[:], out_b[rw:rw + DH + 1, :])
                                for qc in range(NQC):
                                    on_sb[(4 + w, qc)] = recv[
                                        :, qc * 512:(qc + 1) * 512
                                    ]
                        return
                    # local own-half heads (incl. vh7): PE-broadcast flavor
                    for qc in range(NQC):
                        rs = small_pool.tile([1, 512], f32, name="rs", tag="rs", bufs=4)
                        nc.vector.reciprocal_approx_fast(
                            out=rs[:], in_=o_sts[qc][0:1, :]
                        )
                        on = on_pool.tile(
                            [DH + 1, 512], bf16, name="on", tag="on", bufs=8
                        )
                        on_sb[(vh - HL, qc)] = on
                        # row 0 becomes s*(1/s) = 1.0 -> the bias row
                        nc.vector.tensor_tensor(
                            on[:], o_sts[qc][:], pe_broadcast(rs)[:], mult
                        )

                return finish_normalize

            # ---- software-pipelined emission ----
            k_chunk(0, 0)
            q_chunk(0, 0)
            q_chunk(0, 1)
            v_chunk(0)
            v_chunk(1)

            def mk_fillers(vh):
                f = {}

                def addf(slot, fn):
                    f.setdefault(slot, []).append(fn)

                if vh == 0:
                    for nc_ in (1, 2, 3):
                        addf(4 * nc_ - 3, (lambda n=nc_: k_chunk(0, n)))
                    for t in range(2, NT):
                        addf(t - 2, lambda tt=t: v_chunk(tt))
                # remaining heads' K during vh0-2 (kt persists; vh+4 reuses)
                if vh < HL - 1:
                    for i, nc_ in enumerate((0, 1, 2, 3)):
                        addf(2 * i + 2, (lambda hh=vh + 1, n=nc_: k_chunk(hh, n)))
                # next vh's Q
                if vh + 1 < NVH:
                    addf(11, lambda hh=vh + 1: q_chunk(hh, 0))
                    addf(13, lambda hh=vh + 1: q_chunk(hh, 1))
                # projection: ct 0-2 during vh6 (slots {0,1,4,5,6,7}),
                # ct 3-5 during vh7 (slots {0,1,2,4,5,6,7})
                if vh == NVH - 2:
                    for i in range(6):
                        ct, qc = i // 2, i % 2
                        addf(2 * i + 5, lambda c=ct, q=qc: proj_part(c, q))
                if vh == NVH - 1:
                    for i in range(6):
                        ct, qc = 3 + i // 2, i % 2
                        addf(2 * i + 4, lambda c=ct, q=qc: proj_part(c, q))
                return f

            from collections import deque
            pending_norm = deque()
            for vh in range(NVH):
                f = mk_fillers(vh)
                if vh == NVH - 1:
                    # last head: all previous normalizes must land before
                    # the projection fillers (slots 4+)
                    while pending_norm:
                        f.setdefault(2, []).append(pending_norm.popleft())
                elif pending_norm:
                    f.setdefault(3, []).insert(0, pending_norm.popleft())
                pending_norm.append(attn_head(vh, f))

            # ---- tail: vh7 normalize + remaining-slot projection + out ----
            # keep the PE warm through the normalize window so the proj_tail
            # matmuls run at full clock: dummy matmuls gated on vh7's
            # evacuated O' (so the scheduler can't hoist them earlier).
            h7_norm = pending_norm.popleft()
            for _ in range(10):
                wps = psum_o.tile([DH + 1, 512], f32, name="wps", tag="ops")
                nc.tensor.matmul(
                    wps,
                    lhsT=warm_sb[0:DH + 1, 0:DH + 1],
                    rhs=qt_sb[NVH - 1][0:DH + 1, 0:512],
                    start=True,
                    stop=True,
                )
            h7_norm()
            for ct in range(CT):
                proj_tail(ct, use_scalar=(ct % 2 == 1))

    nc.compile()
    return nc


def _get_nc():
    if "nc" not in _CACHE:
        _CACHE["nc"] = _build()
    return _CACHE["nc"]


def _prep_shards(x, w_qkv, w_proj, b_proj):
    bf16 = ml_dtypes.bfloat16
    x = np.asarray(x, dtype=np.float32)
    w_qkv = np.asarray(w_qkv, dtype=np.float32)
    w_proj = np.asarray(w_proj, dtype=np.float32)
    b_proj = np.asarray(b_proj, dtype=np.float32)

    def pmajor(w):  # [768(c), F] -> [128, CT, F] partition-major contiguous
        return np.ascontiguousarray(
            w.reshape(CT, 128, w.shape[1]).transpose(1, 0, 2)
        ).astype(bf16)

    def pad_heads4(w, heads):  # [768(c), 768(f)] -> [768, 4*128] zero-padded
        wp_ = np.zeros((DIM, HL, 128), np.float32)
        wp_[:, :, :DH] = w.reshape(DIM, H, DH)[:, heads, :]
        return wp_.reshape(DIM, HL * 128)

    wq_t = w_qkv[0:DIM].T * SCALE           # [768(c), 768(f)]
    wk_t = w_qkv[DIM:2 * DIM].T
    wv_t = w_qkv[2 * DIM:3 * DIM].T
    wp_heads = w_proj.T.reshape(H, DH, DIM)  # [H, DH, DIM]

    in_maps = []
    for c in range(NCORES):
        b, parity = divmod(c, 2)
        Hs = list(range(0, HL)) if parity == 0 else list(range(HL, H))
        Ho = list(range(HL, H)) if parity == 0 else list(range(0, HL))

        xt = x[b].T  # [768, 2048]
        # arrange columns: [0:1024] = PARTNER's output half, [1024:2048] = OWN
        if parity == 0:
            xt = np.concatenate([xt[:, NQ:], xt[:, :NQ]], axis=1)
        wq_b = pmajor(pad_heads4(wq_t, Hs))
        wk_b = pmajor(pad_heads4(wk_t, Hs))
        wv_b = pmajor(
            np.ascontiguousarray(
                wv_t.reshape(DIM, H, DH)[:, Hs, :]
            ).reshape(DIM, HL * DH)
        )
        # wp slots: j<4 = own head Hs[j], j>=4 = partner head Ho[j-4];
        # row 0 = bias (slot 0 only), rows 1..DH = weights
        wp_arr = np.zeros((DH + 1, H, DIM), np.float32)
        for j, hh in enumerate(Hs + Ho):
            wp_arr[1:DH + 1, j, :] = wp_heads[hh]
        wp_arr[0, 0, :] = b_proj
        # masks: shard A carries the odd core's payload, shard B the even's
        mka = np.full((1, 512), 1.0 if parity == 1 else 0.0, np.float32)
        mkb = np.full((1, 512), 1.0 if parity == 0 else 0.0, np.float32)
        m = {
            f"xt{i}": pmajor(xt[:, i * 512:(i + 1) * 512]) for i in range(4)
        }
        m.update({
            "wq0": np.ascontiguousarray(wq_b[:, :, 0:128]),
            "wqr": np.ascontiguousarray(wq_b[:, :, 128:]),
            "wk0": np.ascontiguousarray(wk_b[:, :, 0:128]),
            "wkr": np.ascontiguousarray(wk_b[:, :, 128:]),
            "wv": wv_b,
            "wp": np.ascontiguousarray(wp_arr).astype(bf16),
            "mka": mka,
            "mkb": mkb,
        })
        in_maps.append(m)
    return in_maps


def kernel(x, w_qkv, w_proj, b_proj):
    from concourse.bass_utils import run_bass_kernel_spmd

    nc = _get_nc()
    in_maps = _prep_shards(x, w_qkv, w_proj, b_proj)
    res = run_bass_kernel_spmd(nc, in_maps, core_ids=list(range(NCORES)))
    out = np.empty((B, N, DIM), np.float32)
    for c in range(NCORES):
        b, half = divmod(c, 2)
        yT = np.asarray(res.results[c]["out"], dtype=np.float32)  # [768, 1024]
        out[b, half * NQ:(half + 1) * NQ, :] = yT.T
    return out


# revision 11
# speedup vs baseline: 1.0217x; 1.0074x over previous
"""Multi-head attention (B=4, N=2048, DIM=768, H=8, DH=96) on 8 TRN2 NeuronCores.

Sharding: (batch, head)-parallel. Core c handles batch c//2 and the 4 heads
Hs = [0..3] (even c) or [4..7] (odd c) — selected host-side by weight slicing,
so the kernel is SPMD-identical. Each core computes K/V/Q for its 4 heads over
ALL 2048 tokens (no K/V duplication — the win vs data-parallel), runs
attention as 8 "virtual heads" vh = (head wh=vh%4, query-half: vh<4 = the
PARTNER's output half, vh>=4 = OWN half), then projects its OWN query half
using all 8 heads' normalized O: 4 local + 4 received from the pair core.

O-exchange: four pairwise ReduceScatters (replica_groups [[0,1],..]), one per
payload head, pipelined against the attention windows. The SPMD read/write
asymmetry is resolved by host-provided 0/1 masks folded into the softmax
normalization: each payload head emits payA = O*(1/s*mA), payB = O*(1/s*mB)
with (mA,mB) = (0,1) on even cores and (1,0) on odd. Shard A rows carry the
odd core's payload, shard B the even's, so each RS sum is EXACTLY the
partner's payload on both sides (own side contributes zeros) — no subtract,
no rounding noise. Queue discipline so nothing ever head-of-line blocks:
staging DMAs ride sync right when payloads exist; the RS trigger (gpsimd)
is a non-blocking doorbell; recv DMAs ride sync ONE window later (after the
next head's staging) so an RS round-trip never delays later staging; the
only consumers of recv tiles are vh6/vh7 projection matmuls, ~2 windows
after the last RS lands.

Per-core compute (all matmuls bf16, fp32 PSUM accumulation):
  - Dense Q^T/K^T projection per head (f-dim zero-padded 96->128), direct
    PSUM->SBUF eviction. K^T [128,2048] per head persists in SBUF (reused by
    both query halves).
  - V projection in natural space; slot 0 of each per-head V tile is a
    constant 1.0 column (1|V) so the attn@V matmul also produces the softmax
    row-sums in PSUM row 0.
  - dots P^T[nk,nq]: lhsT=K^T[96dh, 128nk], rhs=Q^T[96dh, 512nq]; softmax
    scale folded into w_q host-side. Two 512-query chunks land in one
    [128,1024] PSUM pair; ONE exp() on ScalarE PSUM->SBUF(bf16) covers both.
    No max subtraction (logits max ~9, fp32-safe).
  - O'^T[97, nq] accumulated over 16 key tiles; row 0 = row-sum s.
  - normalize, split in two: reciprocal of row 0 (+ mask mults) right after
    the O' eviction at the window end; gpsimd partition_broadcast + multiply
    at slot 3 of the NEXT window (so neither PE nor vector ever waits on a
    cross-engine product; row 0 becomes mA/mB/1.0 = the bias row).
  - proj y^T[c,nq] over 8 O-source slots (0-3 local own-half heads, 4-7
    received partner heads; host permutes wp to match): ct 0-2 accumulate
    slots {0,1,4,5,6,7} during vh6, ct 3-5 accumulate {0,1,2,4,5,6,7} during
    vh7 (bias folded into wp slot-0 row 0); missing slots added at the tail.

Input DMAs are spread by first use across the sync/scalar/vector/gpsimd HW
DGE queues. Output per core: y^T [768, 1024] bf16; host reassembles.
"""

import numpy as np
import ml_dtypes

B, N, DIM = 4, 2048, 768
H, DH = 8, 96
HL = 4             # local heads per core
NQ = N // 2        # query rows per core output
SCALE = DH ** -0.5
NCORES = 8
CT = DIM // 128    # 6 contraction chunks
NT = N // 128      # 16 key tiles
NQC = NQ // 512    # 2 query chunks of 512 per half
NVH = 8            # virtual heads = 4 local heads x 2 query halves

_CACHE = {}


def _build():
    import concourse.mybir as mybir
    import concourse.tile as tile
    from concourse import bacc

    f32 = mybir.dt.float32
    bf16 = mybir.dt.bfloat16
    Exp = mybir.ActivationFunctionType.Exp
    mult = mybir.AluOpType.mult

    nc = bacc.Bacc("TRN2", debug=False, num_devices=NCORES)

    xt_d = [
        nc.dram_tensor(f"xt{i}", [128, CT, 512], bf16, kind="ExternalInput")
        for i in range(4)
    ]
    wq0_d = nc.dram_tensor("wq0", [128, CT, 128], bf16, kind="ExternalInput")
    wk0_d = nc.dram_tensor("wk0", [128, CT, 128], bf16, kind="ExternalInput")
    wqr_d = nc.dram_tensor("wqr", [128, CT, (HL - 1) * 128], bf16, kind="ExternalInput")
    wkr_d = nc.dram_tensor("wkr", [128, CT, (HL - 1) * 128], bf16, kind="ExternalInput")
    wv_d = nc.dram_tensor("wv", [128, CT, HL * DH], bf16, kind="ExternalInput")
    wp_d = nc.dram_tensor("wp", [DH + 1, H, DIM], bf16, kind="ExternalInput")
    mka_d = nc.dram_tensor("mka", [1, 512], f32, kind="ExternalInput")
    mkb_d = nc.dram_tensor("mkb", [1, 512], f32, kind="ExternalInput")
    out_d = nc.dram_tensor("out", [DIM, NQ], bf16, kind="ExternalOutput")

    RG = [[0, 1], [2, 3], [4, 5], [6, 7]]

    with tile.TileContext(nc) as tc:
        with (
            tc.tile_pool(name="const", bufs=1) as cpool,
            tc.tile_pool(name="ptp", bufs=3) as pt_pool,
            tc.tile_pool(name="onp", bufs=14) as on_pool,
            tc.tile_pool(name="smallp", bufs=2) as small_pool,
            tc.tile_pool(name="stagep", bufs=6) as stage_pool,
            tc.tile_pool(name="ysb", bufs=4) as y_pool,
            tc.tile_pool(name="ccd", bufs=1, space="DRAM") as dram_pool,
            tc.tile_pool(name="ps_qkv", bufs=2, space="PSUM") as psum_qkv,
            tc.tile_pool(name="ps_d", bufs=2, space="PSUM") as psum_d,
            tc.tile_pool(name="ps_o", bufs=2, space="PSUM") as psum_o,
        ):
            # ---- persistent SBUF tensors, consolidated input DMAs ----
            xt_sb = [
                cpool.tile([128, CT, 512], bf16, name=f"xt_sb{i}") for i in range(4)
            ]
            wk0_sb = cpool.tile([128, CT, 128], bf16, name="wk0_sb")
            wq0_sb = cpool.tile([128, CT, 128], bf16, name="wq0_sb")
            wkr_sb = cpool.tile([128, CT, (HL - 1) * 128], bf16, name="wkr_sb")
            wqr_sb = cpool.tile([128, CT, (HL - 1) * 128], bf16, name="wqr_sb")
            wv_sb = cpool.tile([128, CT, HL * DH], bf16, name="wv_sb")
            wp_sb = cpool.tile([DH + 1, H, DIM], bf16, name="wp_sb")
            mka_sb = cpool.tile([1, 512], f32, name="mka_sb")
            mkb_sb = cpool.tile([1, 512], f32, name="mkb_sb")
            # K^T per local head, persistent (used by vh and vh+4)
            kt_sb = [cpool.tile([128, N], bf16, name=f"kt{w}") for w in range(HL)]
            qt_sb = {}   # vh -> [128, NQ] tile (rotating)
            v_sb = [cpool.tile([128, HL, DH + 1], bf16, name=f"v{t}") for t in range(NT)]
            y1_sb = [
                [cpool.tile([128, 512], bf16, name=f"y1_{ct}_{qc}") for qc in range(NQC)]
                for ct in range(CT)
            ]

            # collective bounce buffers, one RS per payload head
            in_b = [
                dram_pool.tile(
                    [2 * (DH + 1), NQ], bf16, name=f"ccin{w}", tag=f"ccin{w}"
                )
                for w in range(HL)
            ]
            out_b = [
                dram_pool.tile(
                    [DH + 1, NQ], bf16, name=f"ccout{w}", tag=f"ccout{w}"
                )
                for w in range(HL)
            ]

            # inputs are pre-arranged partition-major on the host: every DMA
            # is a fully-contiguous per-partition transfer. Spread across
            # four HW DGE queues, ordered by first use on each.
            nc.sync.dma_start(wk0_sb[:], wk0_d.ap())
            nc.scalar.dma_start(xt_sb[0][:], xt_d[0].ap())
            nc.sync.dma_start(wq0_sb[:], wq0_d.ap())
            nc.scalar.dma_start(xt_sb[1][:], xt_d[1].ap())
            nc.gpsimd.dma_start(wv_sb[:], wv_d.ap())
            nc.sync.dma_start(xt_sb[2][:], xt_d[2].ap())
            nc.sync.dma_start(xt_sb[3][:], xt_d[3].ap())
            nc.gpsimd.dma_start(wkr_sb[:], wkr_d.ap())
            nc.sync.dma_start(wqr_sb[:], wqr_d.ap())
            nc.gpsimd.dma_start(wp_sb[:], wp_d.ap())
            nc.gpsimd.dma_start(mka_sb[:], mka_d.ap())
            nc.gpsimd.dma_start(mkb_sb[:], mkb_d.ap())

            for t in range(NT):
                nc.vector.memset(v_sb[t][:, :, 0:1], 1.0)

            # identity for re-loading y1 into PSUM at the tail
            id_d = nc.inline_tensor(
                np.eye(128, dtype=ml_dtypes.bfloat16), name="id128"
            )
            id_sb = cpool.tile([128, 128], bf16, name="id_sb")
            nc.sync.dma_start(id_sb[:], id_d.ap())

            # PE warmup: keep the TensorEngine busy through the input-DMA
            # window so the HAM clock gate is at 8/8 when real work starts.
            warm_sb = cpool.tile([128, 128], bf16, name="warm_sb")
            nc.vector.memset(warm_sb[:], 0.0)
            ones_sb = cpool.tile([1, 128], f32, name="ones_sb")
            nc.vector.memset(ones_sb[:], 1.0)
            warm_ps = psum_qkv.tile([128, 128], f32, name="warmps", tag="qkvps")
            for _ in range(40):
                nc.tensor.matmul(
                    warm_ps, lhsT=warm_sb[:], rhs=warm_sb[:], start=True, stop=True
                )

            def xt_cols(lo):
                return xt_sb[lo // 512], lo % 512

            # ---- head-padded Q/K projection chunks (direct eviction) ----
            def k_chunk(wh, nc_):
                src, off = xt_cols(nc_ * 512)
                ps = psum_qkv.tile([128, 512], f32, name="kps", tag="qkvps")
                for ct in range(CT):
                    nc.tensor.matmul(
                        ps,
                        lhsT=(wk0_sb[:, ct, :] if wh == 0
                              else wkr_sb[:, ct, (wh - 1) * 128:wh * 128]),
                        rhs=src[:, ct, off:off + 512],
                        start=(ct == 0),
                        stop=(ct == CT - 1),
                    )
                nc.vector.tensor_copy(
                    out=kt_sb[wh][:, nc_ * 512:(nc_ + 1) * 512], in_=ps[:]
                )

            def q_chunk(vh, qc):
                if qc == 0:
                    qt_sb[vh] = stage_pool.tile(
                        [128, NQ], bf16, name="qt", tag="qt", bufs=6
                    )
                wh = vh % HL
                half = 0 if vh < HL else 1
                src, off = xt_cols(half * NQ + qc * 512)
                ps = psum_qkv.tile([128, 512], f32, name="qps", tag="qkvps")
                for ct in range(CT):
                    nc.tensor.matmul(
                        ps,
                        lhsT=(wq0_sb[:, ct, :] if wh == 0
                              else wqr_sb[:, ct, (wh - 1) * 128:wh * 128]),
                        rhs=src[:, ct, off:off + 512],
                        start=(ct == 0),
                        stop=(ct == CT - 1),
                    )
                nc.vector.tensor_copy(
                    out=qt_sb[vh][:, qc * 512:(qc + 1) * 512], in_=ps[:]
                )

            def v_chunk(t):
                src, off = xt_cols(t * 128)
                ps = psum_qkv.tile([128, 512], f32, name="vps", tag="qkvps")
                vps = ps[:, :HL * DH]
                for ct in range(CT):
                    nc.tensor.matmul(
                        vps,
                        lhsT=src[:, ct, off:off + 128],
                        rhs=wv_sb[:, ct, :],
                        start=(ct == 0),
                        stop=(ct == CT - 1),
                    )
                # single strided eviction into the 4 per-head [*,1:97] slots
                nc.vector.tensor_copy(
                    out=v_sb[t][:, :, 1:DH + 1],
                    in_=vps.rearrange("p (a b) -> p a b", a=HL),
                )

            # O-source slots for the projection: slot j<4 = local own-half
            # head j (vh 4+j), slot 4+w = received partner head (recv[w]).
            on_sb = {}

            def emit_recv(w):
                recv = on_pool.tile(
                    [DH + 1, NQ], bf16, name="recv", tag=f"recv{w}", bufs=1
                )
                nc.sync.dma_start(recv[:], out_b[w][:])
                for qc in range(NQC):
                    on_sb[(4 + w, qc)] = recv[:, qc * 512:(qc + 1) * 512]

            def proj_part(ct, qc):
                """Accumulate available slots; bias folded into slot 0."""
                yp = psum_qkv.tile([128, 512], f32, name="yps", tag="qkvps")
                srcs = [0, 1, 4, 5, 6, 7] if ct < 3 else [0, 1, 2, 4, 5, 6, 7]
                for i, j in enumerate(srcs):
                    nc.tensor.matmul(
                        yp,
                        lhsT=wp_sb[:, j, ct * 128:(ct + 1) * 128],
                        rhs=on_sb[(j, qc)][:],
                        start=(i == 0),
                        stop=(i == len(srcs) - 1),
                    )
                nc.vector.tensor_copy(out=y1_sb[ct][qc][:], in_=yp[:])

            def proj_tail(ct, use_scalar):
                # spread the six ct chunks over all three (now dead) PSUM
                # pools so the chains overlap instead of serializing on one
                # 2-buffer ring.
                tail_srcs = [2, 3] if ct < 3 else [3]
                y_sb = y_pool.tile([128, 1024], bf16, name="y", tag="y")
                if ct % 3 == 0:
                    yp = psum_d.tile([128, 1024], f32, name="yp7", tag="dps")
                    for qc in range(NQC):
                        yps = yp[:, qc * 512:(qc + 1) * 512]
                        nc.tensor.matmul(
                            yps, lhsT=id_sb[:], rhs=y1_sb[ct][qc][:],
                            start=True, stop=False,
                        )
                        for i, j in enumerate(tail_srcs):
                            nc.tensor.matmul(
                                yps,
                                lhsT=wp_sb[:, j, ct * 128:(ct + 1) * 128],
                                rhs=on_sb[(j, qc)][:],
                                start=False,
                                stop=(i == len(tail_srcs) - 1),
                            )
                    if use_scalar:
                        nc.scalar.copy(y_sb[:], yp[:])
                    else:
                        nc.vector.tensor_copy(out=y_sb[:], in_=yp[:])
                else:
                    pool, tag = (
                        (psum_qkv, "qkvps") if ct % 3 == 1 else (psum_o, "ops")
                    )
                    for qc in range(NQC):
                        yps = pool.tile([128, 512], f32, name="yp7n", tag=tag)
                        nc.tensor.matmul(
                            yps, lhsT=id_sb[:], rhs=y1_sb[ct][qc][:],
                            start=True, stop=False,
                        )
                        for i, j in enumerate(tail_srcs):
                            nc.tensor.matmul(
                                yps,
                                lhsT=wp_sb[:, j, ct * 128:(ct + 1) * 128],
                                rhs=on_sb[(j, qc)][:],
                                start=False,
                                stop=(i == len(tail_srcs) - 1),
                            )
                        half = y_sb[:, qc * 512:(qc + 1) * 512]
                        if qc == 0:
                            nc.scalar.copy(half, yps[:])
                        else:
                            nc.vector.tensor_copy(out=half, in_=yps[:])
                dma_eng = nc.sync if ct % 2 == 0 else nc.scalar
                dma_eng.dma_start(
                    out_d.ap()[ct * 128:(ct + 1) * 128, :], y_sb[:]
                )

            # ---- attention for one virtual head, fillers interleaved ----
            def attn_head(vh, fillers):
                wh = vh % HL
                o_ps = [
                    psum_o.tile([DH + 1, 512], f32, name=f"ops{qc}", tag="ops")
                    for qc in range(NQC)
                ]
                for t in range(NT):
                    d_ps = psum_d.tile([128, 1024], f32, name="dps", tag="dps")
                    for qc in range(NQC):
                        nc.tensor.matmul(
                            d_ps[:, qc * 512:(qc + 1) * 512],
                            lhsT=kt_sb[wh][:, t * 128:(t + 1) * 128],
                            rhs=qt_sb[vh][:, qc * 512:(qc + 1) * 512],
                            start=True,
                            stop=True,
                        )
                    pt = pt_pool.tile([128, 1024], bf16, name="pt", tag="pt")
                    nc.scalar.activation(pt[:], d_ps[:], Exp)
                    for qc in range(NQC):
                        nc.tensor.matmul(
                            o_ps[qc],
                            lhsT=v_sb[t][:, wh, :],
                            rhs=pt[:, qc * 512:(qc + 1) * 512],
                            start=(t == 0),
                            stop=(t == NT - 1),
                        )
                    for fn in fillers.get(t, ()):
                        fn()
                # PSUM evacuation + reciprocal (+ payload mask mults) run
                # immediately (frees the o_ps banks; all same-engine local
                # deps). The broadcast+multiply(+staging) half is returned
                # as a closure and emitted at slot 3 of the NEXT window, so
                # no engine ever waits on a cross-engine product.
                o_sts, rss = [], []
                for qc in range(NQC):
                    o_st = small_pool.tile(
                        [DH + 1, 512], f32, name="ostage", tag="ostage", bufs=7
                    )
                    nc.vector.tensor_copy(out=o_st[:], in_=o_ps[qc][:])
                    o_sts.append(o_st)
                if vh == NVH - 1:
                    for qc in range(NQC):
                        rs = small_pool.tile([1, 512], f32, name="rsb", tag="rs", bufs=6)
                        nc.vector.reciprocal_approx_fast(
                            out=rs[:], in_=o_sts[qc][0:1, :]
                        )
                        rss.append(rs)
                elif vh < HL:
                    for qc in range(NQC):
                        rs = small_pool.tile([1, 512], f32, name="rs", tag="rs", bufs=6)
                        nc.vector.reciprocal_approx_fast(
                            out=rs[:], in_=o_sts[qc][0:1, :]
                        )
                        rsA = small_pool.tile([1, 512], f32, name="rsA", tag="rsA", bufs=4)
                        rsB = small_pool.tile([1, 512], f32, name="rsB", tag="rsB", bufs=4)
                        nc.vector.tensor_tensor(rsA[:], rs[:], mka_sb[:], mult)
                        nc.vector.tensor_tensor(rsB[:], rs[:], mkb_sb[:], mult)
                        rss.append((rsA, rsB))
                else:
                    for qc in range(NQC):
                        rs = small_pool.tile([1, 512], f32, name="rs", tag="rs", bufs=6)
                        nc.vector.reciprocal_approx_fast(
                            out=rs[:], in_=o_sts[qc][0:1, :]
                        )
                        rss.append(rs)

                def finish_normalize(vh=vh, wh=wh, o_sts=o_sts, rss=rss):
                    if vh < HL:
                        # partner-destined: masked dual payload, then stage
                        payA = on_pool.tile(
                            [DH + 1, NQ], bf16, name="payA", tag="payA", bufs=4
                        )
                        payB = on_pool.tile(
                            [DH + 1, NQ], bf16, name="payB", tag="payB", bufs=4
                        )
                        for qc in range(NQC):
                            rsA, rsB = rss[qc]
                            sbA = small_pool.tile(
                                [DH + 1, 512], f32, name="sbA", tag="sbc", bufs=4
                            )
                            sbB = small_pool.tile(
                                [DH + 1, 512], f32, name="sbB", tag="sbc", bufs=4
                            )
                            nc.gpsimd.partition_broadcast(sbA[:], rsA[:])
                            nc.gpsimd.partition_broadcast(sbB[:], rsB[:])
                            cs = qc * 512
                            nc.vector.tensor_tensor(
                                payA[:, cs:cs + 512], o_sts[qc][:], sbA[:], mult
                            )
                            nc.vector.tensor_tensor(
                                payB[:, cs:cs + 512], o_sts[qc][:], sbB[:], mult
                            )
                        nc.sync.dma_start(in_b[wh][0:DH + 1, :], payA[:])
                        nc.sync.dma_start(in_b[wh][DH + 1:, :], payB[:])
                        nc.gpsimd.collective_compute(
                            "ReduceScatter",
                            mybir.AluOpType.add,
                            replica_groups=RG,
                            ins=[in_b[wh].opt()],
                            outs=[out_b[wh].opt()],
                        )
                        if wh > 0:
                            emit_recv(wh - 1)
                        return
                    if vh == HL:
                        # the last RS's recv, one window after its staging
                        emit_recv(HL - 1)
                    if vh == NVH - 1:
                        for qc in range(NQC):
                            # PE broadcast (fp32 matmul): the PE is
                            # otherwise idle in the tail window
                            bps = psum_o.tile(
                                [DH + 1, 512], f32, name="bps", tag="ops"
                            )
                            nc.tensor.matmul(
                                bps,
                                lhsT=ones_sb[0:1, 0:DH + 1],
                                rhs=rss[qc][:],
                                start=True,
                                stop=True,
                            )
                            on = on_pool.tile(
                                [DH + 1, 512], bf16, name="on", tag="on", bufs=8
                            )
                            on_sb[(3, qc)] = on
                            nc.vector.tensor_tensor(on[:], o_sts[qc][:], bps[:], mult)
                        return
                    for qc in range(NQC):
                        sb = small_pool.tile(
                            [DH + 1, 512], f32, name="sbc", tag="sbc", bufs=4
                        )
                        nc.gpsimd.partition_broadcast(sb[:], rss[qc][:])
                        on = on_pool.tile(
                            [DH + 1, 512], bf16, name="on", tag="on", bufs=8
                        )
                        on_sb[(vh - HL, qc)] = on
                        # row 0 becomes s*(1/s) = 1.0 -> the bias row
                        nc.vector.tensor_tensor(on[:], o_sts[qc][:], sb[:], mult)

                return finish_normalize

            # ---- software-pipelined emission ----
            k_chunk(0, 0)
            q_chunk(0, 0)
            q_chunk(0, 1)
            v_chunk(0)
            v_chunk(1)

            def mk_fillers(vh):
                f = {}

                def addf(slot, fn):
                    f.setdefault(slot, []).append(fn)

                if vh == 0:
                    for nc_ in (1, 2, 3):
                        addf(4 * nc_ - 3, (lambda n=nc_: k_chunk(0, n)))
                    for t in range(2, NT):
                        addf(t - 2, lambda tt=t: v_chunk(tt))
                # remaining heads' K during vh0-2 (kt persists; vh+4 reuses)
                if vh < HL - 1:
                    for i, nc_ in enumerate((0, 1, 2, 3)):
                        addf(2 * i + 2, (lambda hh=vh + 1, n=nc_: k_chunk(hh, n)))
                # Q chunks: next vh's during vh0-2; vh4+5's during vh3,
                # vh6+7's during vh4 (fills the filler-poor middle windows)
                if vh < HL - 1:
                    addf(11, lambda hh=vh + 1: q_chunk(hh, 0))
                    addf(13, lambda hh=vh + 1: q_chunk(hh, 1))
                elif vh in (HL - 1, HL):
                    b0 = HL + 1 + 2 * (vh - HL + 1)
                    for i, (hh, qc) in enumerate(
                        [(b0 - 1, 0), (b0 - 1, 1), (b0, 0), (b0, 1)]
                    ):
                        addf(11 + i, lambda h=hh, q=qc: q_chunk(h, q))
                # projection: ct 0-2 during vh6 (slots {0,1,4,5,6,7}),
                # ct 3-5 during vh7 (slots {0,1,2,4,5,6,7})
                if vh == NVH - 2:
                    for i in range(6):
                        ct, qc = i // 2, i % 2
                        addf(2 * i + 5, lambda c=ct, q=qc: proj_part(c, q))
                if vh == NVH - 1:
                    for i in range(6):
                        ct, qc = 3 + i // 2, i % 2
                        addf(2 * i + 4, lambda c=ct, q=qc: proj_part(c, q))
                return f

            from collections import deque
            pending_norm = deque()
            for vh in range(NVH):
                f = mk_fillers(vh)
                if vh == NVH - 1:
                    # last head: all previous normalizes must land before
                    # the projection fillers (slots 4+)
                    while pending_norm:
                        f.setdefault(2, []).append(pending_norm.popleft())
                elif pending_norm:
                    f.setdefault(3, []).insert(0, pending_norm.popleft())
                pending_norm.append(attn_head(vh, f))

            # ---- tail: vh7 normalize + remaining-slot projection + out ----
            # keep the PE warm through the normalize window so the proj_tail
            # matmuls run at full clock: dummy matmuls gated on vh7's
            # evacuated O' (so the scheduler can't hoist them earlier).
            h7_norm = pending_norm.popleft()
            for _ in range(10):
                wps = psum_o.tile([DH + 1, 512], f32, name="wps", tag="ops")
                nc.tensor.matmul(
                    wps,
                    lhsT=warm_sb[0:DH + 1, 0:DH + 1],
                    rhs=qt_sb[NVH - 1][0:DH + 1, 0:512],
                    start=True,
                    stop=True,
                )
            h7_norm()
            for ct in range(CT):
                proj_tail(ct, use_scalar=(ct % 2 == 1))

    nc.compile()
    return nc


def _get_nc():
    if "nc" not in _CACHE:
        _CACHE["nc"] = _build()
    return _CACHE["nc"]


def _prep_shards(x, w_qkv, w_proj, b_proj):
    bf16 = ml_dtypes.bfloat16
    x = np.asarray(x, dtype=np.float32)
    w_qkv = np.asarray(w_qkv, dtype=np.float32)
    w_proj = np.asarray(w_proj, dtype=np.float32)
    b_proj = np.asarray(b_proj, dtype=np.float32)

    def pmajor(w):  # [768(c), F] -> [128, CT, F] partition-major contiguous
        return np.ascontiguousarray(
            w.reshape(CT, 128, w.shape[1]).transpose(1, 0, 2)
        ).astype(bf16)

    def pad_heads4(w, heads):  # [768(c), 768(f)] -> [768, 4*128] zero-padded
        wp_ = np.zeros((DIM, HL, 128), np.float32)
        wp_[:, :, :DH] = w.reshape(DIM, H, DH)[:, heads, :]
        return wp_.reshape(DIM, HL * 128)

    wq_t = w_qkv[0:DIM].T * SCALE           # [768(c), 768(f)]
    wk_t = w_qkv[DIM:2 * DIM].T
    wv_t = w_qkv[2 * DIM:3 * DIM].T
    wp_heads = w_proj.T.reshape(H, DH, DIM)  # [H, DH, DIM]

    in_maps = []
    for c in range(NCORES):
        b, parity = divmod(c, 2)
        Hs = list(range(0, HL)) if parity == 0 else list(range(HL, H))
        Ho = list(range(HL, H)) if parity == 0 else list(range(0, HL))

        xt = x[b].T  # [768, 2048]
        # arrange columns: [0:1024] = PARTNER's output half, [1024:2048] = OWN
        if parity == 0:
            xt = np.concatenate([xt[:, NQ:], xt[:, :NQ]], axis=1)
        wq_b = pmajor(pad_heads4(wq_t, Hs))
        wk_b = pmajor(pad_heads4(wk_t, Hs))
        wv_b = pmajor(
            np.ascontiguousarray(
                wv_t.reshape(DIM, H, DH)[:, Hs, :]
            ).reshape(DIM, HL * DH)
        )
        # wp slots: j<4 = own head Hs[j], j>=4 = partner head Ho[j-4];
        # row 0 = bias (slot 0 only), rows 1..DH = weights
        wp_arr = np.zeros((DH + 1, H, DIM), np.float32)
        for j, hh in enumerate(Hs + Ho):
            wp_arr[1:DH + 1, j, :] = wp_heads[hh]
        wp_arr[0, 0, :] = b_proj
        # masks: shard A carries the odd core's payload, shard B the even's
        mka = np.full((1, 512), 1.0 if parity == 1 else 0.0, np.float32)
        mkb = np.full((1, 512), 1.0 if parity == 0 else 0.0, np.float32)
        m = {
            f"xt{i}": pmajor(xt[:, i * 512:(i + 1) * 512]) for i in range(4)
        }
        m.update({
            "wq0": np.ascontiguousarray(wq_b[:, :, 0:128]),
            "wqr": np.ascontiguousarray(wq_b[:, :, 128:]),
            "wk0": np.ascontiguousarray(wk_b[:, :, 0:128]),
            "wkr": np.ascontiguousarray(wk_b[:, :, 128:]),
            "wv": wv_b,
            "wp": np.ascontiguousarray(wp_arr).astype(bf16),
            "mka": mka,
            "mkb": mkb,
        })
        in_maps.append(m)
    return in_maps


def kernel(x, w_qkv, w_proj, b_proj):
    from concourse.bass_utils import run_bass_kernel_spmd

    nc = _get_nc()
    in_maps = _prep_shards(x, w_qkv, w_proj, b_proj)
    res = run_bass_kernel_spmd(nc, in_maps, core_ids=list(range(NCORES)))
    out = np.empty((B, N, DIM), np.float32)
    for c in range(NCORES):
        b, half = divmod(c, 2)
        yT = np.asarray(res.results[c]["out"], dtype=np.float32)  # [768, 1024]
        out[b, half * NQ:(half + 1) * NQ, :] = yT.T
    return out


# revision 17
# speedup vs baseline: 1.0795x; 1.0566x over previous
"""Multi-head attention (B=4, N=2048, DIM=768, H=8, DH=96) on 8 TRN2 NeuronCores.

Sharding: (batch, head)-parallel. Core c handles batch c//2 and the 4 heads
Hs = [0..3] (even c) or [4..7] (odd c) — selected host-side by weight slicing,
so the kernel is SPMD-identical. Each core computes K/V/Q for its 4 heads over
ALL 2048 tokens (no K/V duplication — the win vs data-parallel), runs
attention as 8 "virtual heads" vh = (head wh=vh%4, query-half: vh<4 = the
PARTNER's output half, vh>=4 = OWN half), then projects its OWN query half
using all 8 heads' normalized O: 4 local + 4 received from the pair core.

O-exchange: four pairwise ReduceScatters (replica_groups [[0,1],..]), one per
payload head, pipelined against the attention windows. The SPMD read/write
asymmetry is resolved by host-provided 0/1 masks folded into the softmax
normalization: each payload head emits payA = O*(1/s*mA), payB = O*(1/s*mB)
with (mA,mB) = (0,1) on even cores and (1,0) on odd. Shard A rows carry the
odd core's payload, shard B the even's, so each RS sum is EXACTLY the
partner's payload on both sides (own side contributes zeros) — no subtract,
no rounding noise. Queue discipline so nothing ever head-of-line blocks:
staging DMAs ride sync right when payloads exist; the RS trigger (gpsimd)
is a non-blocking doorbell; recv DMAs ride sync ONE window later (after the
next head's staging) so an RS round-trip never delays later staging; the
only consumers of recv tiles are vh6/vh7 projection matmuls, ~2 windows
after the last RS lands.

Per-core compute (all matmuls bf16, fp32 PSUM accumulation):
  - Dense Q^T/K^T projection per head (f-dim zero-padded 96->128), direct
    PSUM->SBUF eviction. K^T [128,2048] per head persists in SBUF (reused by
    both query halves).
  - V projection in natural space; slot 0 of each per-head V tile is a
    constant 1.0 column (1|V) so the attn@V matmul also produces the softmax
    row-sums in PSUM row 0.
  - dots P^T[nk,nq]: lhsT=K^T[96dh, 128nk], rhs=Q^T[96dh, 512nq]; softmax
    scale folded into w_q host-side. Two 512-query chunks land in one
    [128,1024] PSUM pair; ONE exp() on ScalarE PSUM->SBUF(bf16) covers both.
    No max subtraction (logits max ~9, fp32-safe).
  - O'^T[97, nq] accumulated over 16 key tiles; row 0 = row-sum s.
  - normalize, split in two: reciprocal of row 0 (+ mask mults) right after
    the O' eviction at the window end; gpsimd partition_broadcast + multiply
    at slot 3 of the NEXT window (so neither PE nor vector ever waits on a
    cross-engine product; row 0 becomes mA/mB/1.0 = the bias row).
  - proj y^T[c,nq] over 8 O-source slots (0-3 local own-half heads, 4-7
    received partner heads; host permutes wp to match): ct 0-2 accumulate
    slots {0,1,4,5,6,7} during vh6, ct 3-5 accumulate {0,1,2,4,5,6,7} during
    vh7 (bias folded into wp slot-0 row 0); missing slots added at the tail.

Input DMAs are spread by first use across the sync/scalar/vector/gpsimd HW
DGE queues. Output per core: y^T [768, 1024] bf16; host reassembles.
"""

import numpy as np
import ml_dtypes

B, N, DIM = 4, 2048, 768
H, DH = 8, 96
HL = 4             # local heads per core
NQ = N // 2        # query rows per core output
SCALE = DH ** -0.5
NCORES = 8
CT = DIM // 128    # 6 contraction chunks
NT = N // 128      # 16 key tiles
NQC = NQ // 512    # 2 query chunks of 512 per half
NVH = 8            # virtual heads = 4 local heads x 2 query halves

_CACHE = {}


def _build():
    import concourse.mybir as mybir
    import concourse.tile as tile
    from concourse import bacc

    f32 = mybir.dt.float32
    bf16 = mybir.dt.bfloat16
    Exp = mybir.ActivationFunctionType.Exp
    mult = mybir.AluOpType.mult

    nc = bacc.Bacc("TRN2", debug=False, num_devices=NCORES)

    xt_d = [
        nc.dram_tensor(f"xt{i}", [128, CT, 512], bf16, kind="ExternalInput")
        for i in range(4)
    ]
    wq0_d = nc.dram_tensor("wq0", [128, CT, 128], bf16, kind="ExternalInput")
    wk0_d = nc.dram_tensor("wk0", [128, CT, 128], bf16, kind="ExternalInput")
    wqr_d = nc.dram_tensor("wqr", [128, CT, (HL - 1) * 128], bf16, kind="ExternalInput")
    wkr_d = nc.dram_tensor("wkr", [128, CT, (HL - 1) * 128], bf16, kind="ExternalInput")
    wv_d = nc.dram_tensor("wv", [128, CT, HL * DH], bf16, kind="ExternalInput")
    wp_d = nc.dram_tensor("wp", [DH + 1, H, DIM], bf16, kind="ExternalInput")
    mka_d = nc.dram_tensor("mka", [1, 512], f32, kind="ExternalInput")
    mkb_d = nc.dram_tensor("mkb", [1, 512], f32, kind="ExternalInput")
    out_d = nc.dram_tensor("out", [DIM, NQ], bf16, kind="ExternalOutput")

    RG = [[0, 1], [2, 3], [4, 5], [6, 7]]

    with tile.TileContext(nc) as tc:
        with (
            tc.tile_pool(name="const", bufs=1) as cpool,
            tc.tile_pool(name="ptp", bufs=3) as pt_pool,
            tc.tile_pool(name="onp", bufs=14) as on_pool,
            tc.tile_pool(name="smallp", bufs=2) as small_pool,
            tc.tile_pool(name="stagep", bufs=6) as stage_pool,
            tc.tile_pool(name="ysb", bufs=4) as y_pool,
            tc.tile_pool(name="ccd", bufs=1, space="DRAM") as dram_pool,
            tc.tile_pool(name="ps_qkv", bufs=2, space="PSUM") as psum_qkv,
            tc.tile_pool(name="ps_d", bufs=2, space="PSUM") as psum_d,
            tc.tile_pool(name="ps_o", bufs=2, space="PSUM") as psum_o,
        ):
            # ---- persistent SBUF tensors, consolidated input DMAs ----
            xt_sb = [
                cpool.tile([128, CT, 512], bf16, name=f"xt_sb{i}") for i in range(4)
            ]
            wk0_sb = cpool.tile([128, CT, 128], bf16, name="wk0_sb")
            wq0_sb = cpool.tile([128, CT, 128], bf16, name="wq0_sb")
            wkr_sb = cpool.tile([128, CT, (HL - 1) * 128], bf16, name="wkr_sb")
            wqr_sb = cpool.tile([128, CT, (HL - 1) * 128], bf16, name="wqr_sb")
            wv_sb = cpool.tile([128, CT, HL * DH], bf16, name="wv_sb")
            wp_sb = cpool.tile([DH + 1, H, DIM], bf16, name="wp_sb")
            mka_sb = cpool.tile([1, 512], f32, name="mka_sb")
            mkb_sb = cpool.tile([1, 512], f32, name="mkb_sb")
            # K^T per local head, persistent (used by vh and vh+4)
            kt_sb = [cpool.tile([128, N], bf16, name=f"kt{w}") for w in range(HL)]
            qt_sb = {}   # vh -> [128, NQ] tile (rotating)
            v_sb = [cpool.tile([128, HL, DH + 1], bf16, name=f"v{t}") for t in range(NT)]
            y1_sb = [
                [cpool.tile([128, 512], bf16, name=f"y1_{ct}_{qc}") for qc in range(NQC)]
                for ct in range(CT)
            ]

            # collective bounce buffers: ONE RS for all 4 payload heads
            # (each ncfw collective op costs ~38us of control latency here,
            # so batching into one op beats pipelining four). Shard A rows
            # [0:388] = payA blocks (odd core's payload), shard B rows
            # [388:776] = payB (even core's).
            in_b = dram_pool.tile(
                [2 * HL * (DH + 1), NQ], bf16, name="ccin", tag="ccin"
            )
            out_b = dram_pool.tile(
                [HL * (DH + 1), NQ], bf16, name="ccout", tag="ccout"
            )

            # inputs are pre-arranged partition-major on the host: every DMA
            # is a fully-contiguous per-partition transfer. Spread across
            # four HW DGE queues, ordered by first use on each.
            nc.sync.dma_start(wk0_sb[:], wk0_d.ap())
            nc.scalar.dma_start(xt_sb[0][:], xt_d[0].ap())
            nc.sync.dma_start(wq0_sb[:], wq0_d.ap())
            nc.scalar.dma_start(xt_sb[1][:], xt_d[1].ap())
            nc.gpsimd.dma_start(wv_sb[:], wv_d.ap())
            nc.sync.dma_start(xt_sb[2][:], xt_d[2].ap())
            nc.sync.dma_start(xt_sb[3][:], xt_d[3].ap())
            nc.gpsimd.dma_start(wkr_sb[:], wkr_d.ap())
            nc.sync.dma_start(wqr_sb[:], wqr_d.ap())
            nc.gpsimd.dma_start(wp_sb[:], wp_d.ap())
            nc.gpsimd.dma_start(mka_sb[:], mka_d.ap())
            nc.gpsimd.dma_start(mkb_sb[:], mkb_d.ap())

            for t in range(NT):
                nc.vector.memset(v_sb[t][:, :, 0:1], 1.0)

            # identity for re-loading y1 into PSUM at the tail
            id_d = nc.inline_tensor(
                np.eye(128, dtype=ml_dtypes.bfloat16), name="id128"
            )
            id_sb = cpool.tile([128, 128], bf16, name="id_sb")
            nc.sync.dma_start(id_sb[:], id_d.ap())

            # PE warmup: keep the TensorEngine busy through the input-DMA
            # window so the HAM clock gate is at 8/8 when real work starts.
            warm_sb = cpool.tile([128, 128], bf16, name="warm_sb")
            nc.vector.memset(warm_sb[:], 0.0)
            ones_sb = cpool.tile([1, 128], f32, name="ones_sb")
            nc.vector.memset(ones_sb[:], 1.0)
            warm_ps = psum_qkv.tile([128, 128], f32, name="warmps", tag="qkvps")
            for _ in range(40):
                nc.tensor.matmul(
                    warm_ps, lhsT=warm_sb[:], rhs=warm_sb[:], start=True, stop=True
                )

            def xt_cols(lo):
                return xt_sb[lo // 512], lo % 512

            # ---- head-padded Q/K projection chunks (direct eviction) ----
            def k_chunk(wh, nc_):
                src, off = xt_cols(nc_ * 512)
                ps = psum_qkv.tile([128, 512], f32, name="kps", tag="qkvps")
                for ct in range(CT):
                    nc.tensor.matmul(
                        ps,
                        lhsT=(wk0_sb[:, ct, :] if wh == 0
                              else wkr_sb[:, ct, (wh - 1) * 128:wh * 128]),
                        rhs=src[:, ct, off:off + 512],
                        start=(ct == 0),
                        stop=(ct == CT - 1),
                    )
                nc.vector.tensor_copy(
                    out=kt_sb[wh][:, nc_ * 512:(nc_ + 1) * 512], in_=ps[:]
                )

            def q_chunk(vh, qc):
                if qc == 0:
                    qt_sb[vh] = stage_pool.tile(
                        [128, NQ], bf16, name="qt", tag="qt", bufs=6
                    )
                wh = vh % HL
                half = 0 if vh < HL else 1
                src, off = xt_cols(half * NQ + qc * 512)
                ps = psum_qkv.tile([128, 512], f32, name="qps", tag="qkvps")
                for ct in range(CT):
                    nc.tensor.matmul(
                        ps,
                        lhsT=(wq0_sb[:, ct, :] if wh == 0
                              else wqr_sb[:, ct, (wh - 1) * 128:wh * 128]),
                        rhs=src[:, ct, off:off + 512],
                        start=(ct == 0),
                        stop=(ct == CT - 1),
                    )
                nc.vector.tensor_copy(
                    out=qt_sb[vh][:, qc * 512:(qc + 1) * 512], in_=ps[:]
                )

            def v_chunk(t):
                src, off = xt_cols(t * 128)
                ps = psum_qkv.tile([128, 512], f32, name="vps", tag="qkvps")
                vps = ps[:, :HL * DH]
                for ct in range(CT):
                    nc.tensor.matmul(
                        vps,
                        lhsT=src[:, ct, off:off + 128],
                        rhs=wv_sb[:, ct, :],
                        start=(ct == 0),
                        stop=(ct == CT - 1),
                    )
                # single strided eviction into the 4 per-head [*,1:97] slots
                nc.vector.tensor_copy(
                    out=v_sb[t][:, :, 1:DH + 1],
                    in_=vps.rearrange("p (a b) -> p a b", a=HL),
                )

            # O-source slots for the projection: slot j<4 = local own-half
            # head j (vh 4+j), slot 4+w = received partner head (recv[w]).
            on_sb = {}

            def emit_recv(w):
                recv = on_pool.tile(
                    [DH + 1, NQ], bf16, name="recv", tag=f"recv{w}", bufs=1
                )
                rw = w * (DH + 1)
                nc.sync.dma_start(recv[:], out_b[rw:rw + DH + 1, :])
                for qc in range(NQC):
                    on_sb[(4 + w, qc)] = recv[:, qc * 512:(qc + 1) * 512]

            def proj_part(ct, qc):
                """Accumulate slots {0,1,2,4,5,6,7}; bias folded in slot 0."""
                yp = psum_qkv.tile([128, 512], f32, name="yps", tag="qkvps")
                srcs = [0, 1, 2, 4, 5, 6, 7]
                for i, j in enumerate(srcs):
                    nc.tensor.matmul(
                        yp,
                        lhsT=wp_sb[:, j, ct * 128:(ct + 1) * 128],
                        rhs=on_sb[(j, qc)][:],
                        start=(i == 0),
                        stop=(i == len(srcs) - 1),
                    )
                nc.vector.tensor_copy(out=y1_sb[ct][qc][:], in_=yp[:])

            def proj_tail(ct, use_scalar):
                # spread the six ct chunks over all three (now dead) PSUM
                # pools so the chains overlap instead of serializing on one
                # 2-buffer ring.
                tail_srcs = [3]
                y_sb = y_pool.tile([128, 1024], bf16, name="y", tag="y")
                if ct % 3 == 0:
                    yp = psum_d.tile([128, 1024], f32, name="yp7", tag="dps")
                    for qc in range(NQC):
                        yps = yp[:, qc * 512:(qc + 1) * 512]
                        nc.tensor.matmul(
                            yps, lhsT=id_sb[:], rhs=y1_sb[ct][qc][:],
                            start=True, stop=False,
                        )
                        for i, j in enumerate(tail_srcs):
                            nc.tensor.matmul(
                                yps,
                                lhsT=wp_sb[:, j, ct * 128:(ct + 1) * 128],
                                rhs=on_sb[(j, qc)][:],
                                start=False,
                                stop=(i == len(tail_srcs) - 1),
                            )
                    if use_scalar:
                        nc.scalar.copy(y_sb[:], yp[:])
                    else:
                        nc.vector.tensor_copy(out=y_sb[:], in_=yp[:])
                else:
                    pool, tag = (
                        (psum_qkv, "qkvps") if ct % 3 == 1 else (psum_o, "ops")
                    )
                    for qc in range(NQC):
                        yps = pool.tile([128, 512], f32, name="yp7n", tag=tag)
                        nc.tensor.matmul(
                            yps, lhsT=id_sb[:], rhs=y1_sb[ct][qc][:],
                            start=True, stop=False,
                        )
                        for i, j in enumerate(tail_srcs):
                            nc.tensor.matmul(
                                yps,
                                lhsT=wp_sb[:, j, ct * 128:(ct + 1) * 128],
                                rhs=on_sb[(j, qc)][:],
                                start=False,
                                stop=(i == len(tail_srcs) - 1),
                            )
                        half = y_sb[:, qc * 512:(qc + 1) * 512]
                        if qc == 0:
                            nc.scalar.copy(half, yps[:])
                        else:
                            nc.vector.tensor_copy(out=half, in_=yps[:])
                dma_eng = nc.sync if ct % 2 == 0 else nc.scalar
                dma_eng.dma_start(
                    out_d.ap()[ct * 128:(ct + 1) * 128, :], y_sb[:]
                )

            # ---- attention for one virtual head, fillers interleaved ----
            def attn_head(vh, fillers):
                wh = vh % HL
                o_ps = [
                    psum_o.tile([DH + 1, 512], f32, name=f"ops{qc}", tag="ops")
                    for qc in range(NQC)
                ]
                for t in range(NT):
                    d_ps = psum_d.tile([128, 1024], f32, name="dps", tag="dps")
                    for qc in range(NQC):
                        nc.tensor.matmul(
                            d_ps[:, qc * 512:(qc + 1) * 512],
                            lhsT=kt_sb[wh][:, t * 128:(t + 1) * 128],
                            rhs=qt_sb[vh][:, qc * 512:(qc + 1) * 512],
                            start=True,
                            stop=True,
                        )
                    pt = pt_pool.tile([128, 1024], bf16, name="pt", tag="pt")
                    nc.scalar.activation(pt[:], d_ps[:], Exp)
                    for qc in range(NQC):
                        nc.tensor.matmul(
                            o_ps[qc],
                            lhsT=v_sb[t][:, wh, :],
                            rhs=pt[:, qc * 512:(qc + 1) * 512],
                            start=(t == 0),
                            stop=(t == NT - 1),
                        )
                    for fn in fillers.get(t, ()):
                        fn()
                # PSUM evacuation + reciprocal (+ payload mask mults) run
                # immediately (frees the o_ps banks; all same-engine local
                # deps). The broadcast+multiply(+staging) half is returned
                # as a closure and emitted at slot 3 of the NEXT window, so
                # no engine ever waits on a cross-engine product.
                o_sts, rss = [], []
                for qc in range(NQC):
                    o_st = small_pool.tile(
                        [DH + 1, 512], f32, name="ostage", tag="ostage", bufs=7
                    )
                    nc.vector.tensor_copy(out=o_st[:], in_=o_ps[qc][:])
                    o_sts.append(o_st)
                if vh == NVH - 1:
                    for qc in range(NQC):
                        rs = small_pool.tile([1, 512], f32, name="rsb", tag="rs", bufs=6)
                        nc.vector.reciprocal_approx_fast(
                            out=rs[:], in_=o_sts[qc][0:1, :]
                        )
                        rss.append(rs)
                elif vh < HL:
                    for qc in range(NQC):
                        rs = small_pool.tile([1, 512], f32, name="rs", tag="rs", bufs=6)
                        nc.vector.reciprocal_approx_fast(
                            out=rs[:], in_=o_sts[qc][0:1, :]
                        )
                        rsA = small_pool.tile([1, 512], f32, name="rsA", tag="rsA", bufs=4)
                        rsB = small_pool.tile([1, 512], f32, name="rsB", tag="rsB", bufs=4)
                        nc.vector.tensor_tensor(rsA[:], rs[:], mka_sb[:], mult)
                        nc.vector.tensor_tensor(rsB[:], rs[:], mkb_sb[:], mult)
                        rss.append((rsA, rsB))
                else:
                    for qc in range(NQC):
                        rs = small_pool.tile([1, 512], f32, name="rs", tag="rs", bufs=6)
                        nc.vector.reciprocal_approx_fast(
                            out=rs[:], in_=o_sts[qc][0:1, :]
                        )
                        rss.append(rs)

                def finish_normalize(vh=vh, wh=wh, o_sts=o_sts, rss=rss):
                    if vh < HL:
                        # partner-destined: masked dual payload, then stage
                        payA = on_pool.tile(
                            [DH + 1, NQ], bf16, name="payA", tag="payA", bufs=4
                        )
                        payB = on_pool.tile(
                            [DH + 1, NQ], bf16, name="payB", tag="payB", bufs=4
                        )
                        for qc in range(NQC):
                            rsA, rsB = rss[qc]
                            sbA = small_pool.tile(
                                [DH + 1, 512], f32, name="sbA", tag="sbc", bufs=4
                            )
                            sbB = small_pool.tile(
                                [DH + 1, 512], f32, name="sbB", tag="sbc", bufs=4
                            )
                            nc.gpsimd.partition_broadcast(sbA[:], rsA[:])
                            nc.gpsimd.partition_broadcast(sbB[:], rsB[:])
                            cs = qc * 512
                            nc.vector.tensor_tensor(
                                payA[:, cs:cs + 512], o_sts[qc][:], sbA[:], mult
                            )
                            nc.vector.tensor_tensor(
                                payB[:, cs:cs + 512], o_sts[qc][:], sbB[:], mult
                            )
                        r0 = wh * (DH + 1)
                        nc.sync.dma_start(in_b[r0:r0 + DH + 1, :], payA[:])
                        r1 = HL * (DH + 1) + r0
                        nc.sync.dma_start(in_b[r1:r1 + DH + 1, :], payB[:])
                        if wh == HL - 1:
                            # all 8 payload blocks staged -> ONE collective;
                            # recv DMAs follow on sync (idle until the tail)
                            nc.gpsimd.collective_compute(
                                "ReduceScatter",
                                mybir.AluOpType.add,
                                replica_groups=RG,
                                ins=[in_b.opt()],
                                outs=[out_b.opt()],
                            )
                            for w in range(HL):
                                emit_recv(w)
                        return
                    if vh == NVH - 1:
                        for qc in range(NQC):
                            # PE broadcast (fp32 matmul): the PE is
                            # otherwise idle in the tail window
                            bps = psum_o.tile(
                                [DH + 1, 512], f32, name="bps", tag="ops"
                            )
                            nc.tensor.matmul(
                                bps,
                                lhsT=ones_sb[0:1, 0:DH + 1],
                                rhs=rss[qc][:],
                                start=True,
                                stop=True,
                            )
                            on = on_pool.tile(
                                [DH + 1, 512], bf16, name="on", tag="on", bufs=8
                            )
                            on_sb[(3, qc)] = on
                            nc.vector.tensor_tensor(on[:], o_sts[qc][:], bps[:], mult)
                        return
                    for qc in range(NQC):
                        sb = small_pool.tile(
                            [DH + 1, 512], f32, name="sbc", tag="sbc", bufs=4
                        )
                        nc.gpsimd.partition_broadcast(sb[:], rss[qc][:])
                        on = on_pool.tile(
                            [DH + 1, 512], bf16, name="on", tag="on", bufs=8
                        )
                        on_sb[(vh - HL, qc)] = on
                        # row 0 becomes s*(1/s) = 1.0 -> the bias row
                        nc.vector.tensor_tensor(on[:], o_sts[qc][:], sb[:], mult)

                return finish_normalize

            # ---- software-pipelined emission ----
            k_chunk(0, 0)
            q_chunk(0, 0)
            q_chunk(0, 1)
            v_chunk(0)
            v_chunk(1)

            def mk_fillers(vh):
                f = {}

                def addf(slot, fn):
                    f.setdefault(slot, []).append(fn)

                if vh == 0:
                    for nc_ in (1, 2, 3):
                        addf(4 * nc_ - 3, (lambda n=nc_: k_chunk(0, n)))
                    for t in range(2, NT):
                        addf(t - 2, lambda tt=t: v_chunk(tt))
                # remaining heads' K during vh0-2 (kt persists; vh+4 reuses)
                if vh < HL - 1:
                    for i, nc_ in enumerate((0, 1, 2, 3)):
                        addf(2 * i + 2, (lambda hh=vh + 1, n=nc_: k_chunk(hh, n)))
                # Q chunks: next vh's during vh0-2; vh4+5's during vh3,
                # vh6+7's during vh4 (fills the filler-poor middle windows)
                if vh < HL - 1:
                    addf(11, lambda hh=vh + 1: q_chunk(hh, 0))
                    addf(13, lambda hh=vh + 1: q_chunk(hh, 1))
                elif vh in (HL - 1, HL):
                    b0 = HL + 1 + 2 * (vh - HL + 1)
                    for i, (hh, qc) in enumerate(
                        [(b0 - 1, 0), (b0 - 1, 1), (b0, 0), (b0, 1)]
                    ):
                        addf(11 + i, lambda h=hh, q=qc: q_chunk(h, q))
                # projection: all 12 chunks during vh7 (slots {0,1,2,4-7};
                # the RS lands ~1.5 windows earlier, so no stall)
                if vh == NVH - 1:
                    for i in range(12):
                        ct, qc = i // 2, i % 2
                        addf(i + 3, lambda c=ct, q=qc: proj_part(c, q))
                return f

            from collections import deque
            pending_norm = deque()
            for vh in range(NVH):
                f = mk_fillers(vh)
                if vh == NVH - 1:
                    # last head: all previous normalizes must land before
                    # the projection fillers (slots 4+)
                    while pending_norm:
                        f.setdefault(2, []).append(pending_norm.popleft())
                elif pending_norm:
                    f.setdefault(3, []).insert(0, pending_norm.popleft())
                pending_norm.append(attn_head(vh, f))

            # ---- tail: vh7 normalize + remaining-slot projection + out ----
            # keep the PE warm through the normalize window so the proj_tail
            # matmuls run at full clock: dummy matmuls gated on vh7's
            # evacuated O' (so the scheduler can't hoist them earlier).
            h7_norm = pending_norm.popleft()
            for _ in range(10):
                wps = psum_o.tile([DH + 1, 512], f32, name="wps", tag="ops")
                nc.tensor.matmul(
                    wps,
                    lhsT=warm_sb[0:DH + 1, 0:DH + 1],
                    rhs=qt_sb[NVH - 1][0:DH + 1, 0:512],
                    start=True,
                    stop=True,
                )
            h7_norm()
            for ct in range(CT):
                proj_tail(ct, use_scalar=(ct % 2 == 1))

    nc.compile()
    return nc


def _get_nc():
    if "nc" not in _CACHE:
        _CACHE["nc"] = _build()
    return _CACHE["nc"]


def _prep_shards(x, w_qkv, w_proj, b_proj):
    bf16 = ml_dtypes.bfloat16
    x = np.asarray(x, dtype=np.float32)
    w_qkv = np.asarray(w_qkv, dtype=np.float32)
    w_proj = np.asarray(w_proj, dtype=np.float32)
    b_proj = np.asarray(b_proj, dtype=np.float32)

    def pmajor(w):  # [768(c), F] -> [128, CT, F] partition-major contiguous
        return np.ascontiguousarray(
            w.reshape(CT, 128, w.shape[1]).transpose(1, 0, 2)
        ).astype(bf16)

    def pad_heads4(w, heads):  # [768(c), 768(f)] -> [768, 4*128] zero-padded
        wp_ = np.zeros((DIM, HL, 128), np.float32)
        wp_[:, :, :DH] = w.reshape(DIM, H, DH)[:, heads, :]
        return wp_.reshape(DIM, HL * 128)

    wq_t = w_qkv[0:DIM].T * SCALE           # [768(c), 768(f)]
    wk_t = w_qkv[DIM:2 * DIM].T
    wv_t = w_qkv[2 * DIM:3 * DIM].T
    wp_heads = w_proj.T.reshape(H, DH, DIM)  # [H, DH, DIM]

    in_maps = []
    for c in range(NCORES):
        b, parity = divmod(c, 2)
        Hs = list(range(0, HL)) if parity == 0 else list(range(HL, H))
        Ho = list(range(HL, H)) if parity == 0 else list(range(0, HL))

        xt = x[b].T  # [768, 2048]
        # arrange columns: [0:1024] = PARTNER's output half, [1024:2048] = OWN
        if parity == 0:
            xt = np.concatenate([xt[:, NQ:], xt[:, :NQ]], axis=1)
        wq_b = pmajor(pad_heads4(wq_t, Hs))
        wk_b = pmajor(pad_heads4(wk_t, Hs))
        wv_b = pmajor(
            np.ascontiguousarray(
                wv_t.reshape(DIM, H, DH)[:, Hs, :]
            ).reshape(DIM, HL * DH)
        )
        # wp slots: j<4 = own head Hs[j], j>=4 = partner head Ho[j-4];
        # row 0 = bias (slot 0 only), rows 1..DH = weights
        wp_arr = np.zeros((DH + 1, H, DIM), np.float32)
        for j, hh in enumerate(Hs + Ho):
            wp_arr[1:DH + 1, j, :] = wp_heads[hh]
        wp_arr[0, 0, :] = b_proj
        # masks: shard A carries the odd core's payload, shard B the even's
        mka = np.full((1, 512), 1.0 if parity == 1 else 0.0, np.float32)
        mkb = np.full((1, 512), 1.0 if parity == 0 else 0.0, np.float32)
        m = {
            f"xt{i}": pmajor(xt[:, i * 512:(i + 1) * 512]) for i in range(4)
        }
        m.update({
            "wq0": np.ascontiguousarray(wq_b[:, :, 0:128]),
            "wqr": np.ascontiguousarray(wq_b[:, :, 128:]),
            "wk0": np.ascontiguousarray(wk_b[:, :, 0:128]),
            "wkr": np.ascontiguousarray(wk_b[:, :, 128:]),
            "wv": wv_b,
            "wp": np.ascontiguousarray(wp_arr).astype(bf16),
            "mka": mka,
            "mkb": mkb,
        })
        in_maps.append(m)
    return in_maps


def kernel(x, w_qkv, w_proj, b_proj):
    from concourse.bass_utils import run_bass_kernel_spmd

    nc = _get_nc()
    in_maps = _prep_shards(x, w_qkv, w_proj, b_proj)
    res = run_bass_kernel_spmd(nc, in_maps, core_ids=list(range(NCORES)))
    out = np.empty((B, N, DIM), np.float32)
    for c in range(NCORES):
        b, half = divmod(c, 2)
        yT = np.asarray(res.results[c]["out"], dtype=np.float32)  # [768, 1024]
        out[b, half * NQ:(half + 1) * NQ, :] = yT.T
    return out
